# revision 27
# baseline (speedup 1.0000x reference)
"""Trainium2 Bass kernel for nn_DeformableConvStandard.

The deformable interpolation + both convs are linear in `inp` once the
(tiny) offsets are known, so the whole module collapses to

    out = Pt + Pd * sigmoid(ctrl' @ W),   Pt = X @ A_t,  Pd = X @ D

with A_t, D: [48, 12] host-built from offsets/conv weights, and the gate
bias pre-folded into ctrl' = ctrl + bparam @ W^-1 on the host. Rows of
[A_t|D] that are identically zero (deform positions never sampled) are
pruned from the streamed X features.

Per-core layout (16 batches = 8 pairs of 2): each pair's x-path runs in
two 2-bank PSUM tiles [112, 1024] (cols = batch), one per group-half h,
with Pt at partitions 0-47 and Pd at 64-111 (48-part blocks may only
start at 0/64). The streamed features split by weight sensitivity into
12 fp16 ("hi") + 32 fp8-e3m4 ("lo") contraction chunks, which cuts the
dominant DMA stream by 30% at ~0.9e-2 rel err. The combine
out = Pt + Pd*S is spread over all four compute engines per pair:
  - one merged DVE mult [48,1024] for half h0, two PE identity-add
    matmuls, one merged ACT copy [48,1024] to the fp16 out tile;
  - quarter C (batch0, h1) fully on DVE ([48,512] mult+add);
  - quarter D (batch1, h1): ACT copies the PSUM tile to SBUF, then the
    (otherwise idle) GPSIMD engine does mult+add SBUF-only.
One merged sigmoid [112,1024] per pair (gate PSUM tile spans 2 banks).
All input DMAs are hoisted up-front on the sync queue; output stores
follow on the same queue (2-pair output tiles, last 2 pairs split for a
shorter tail).
"""
import numpy as np
from contextlib import ExitStack

N_CORES = 8
B, NNODES = 128, 4096
NDW, LL, NPRED, NDRIFT = 3, 16, 12, 2
BPC = B // N_CORES          # batches per core: 16
NG = NNODES // 512          # node groups per batch: 8
N_HI = 12                   # fp16-streamed features (4*N_HI <= 128)
N_LO = 32                   # fp8e3m4-streamed features (4*N_LO <= 128)
MAX_W_COND = 1e4            # fold bias into ctrl only if W is this well-posed
N_WARM_MM = 10              # dummy PE warm-up matmuls (p-state ramp + fill)
WARM_FREE = 400             # free size of each warm-up matmul


def _build_A(offset, conv_w, mode):
    """A [48, 12] with pred = X @ A for X [rows, 48], feature = d*16+l."""
    off = np.asarray(offset, np.float32)
    pos = np.tanh(off) * np.float32(NDRIFT) + (
        np.arange(NPRED, dtype=np.float32) + np.float32(NDRIFT)
    )[None, :]
    key = np.floor(pos)
    frac = (pos - key).astype(np.float64)
    idx = key.astype(np.int32)
    M = np.zeros((NDW, LL, NPRED), np.float64)
    for d in range(NDW):
        for j in range(NPRED):
            M[d, idx[d, j], j] += 1.0 - frac[d, j]
            M[d, idx[d, j] + 1, j] += frac[d, j]
    A = np.zeros((NDW, LL, NPRED), np.float64)
    w = np.asarray(conv_w, np.float64)
    if mode == "t":
        for p in range(NPRED):
            for k in range(3):
                j = p + k - 1
                if 0 <= j < NPRED:
                    A[:, :, p] += w[0, :, k][:, None] * M[:, :, j]
    else:
        for o in range(NPRED):
            for d in range(NDW):
                for c in range(NPRED):
                    A[d, :, o] += w[o, c, d] * M[d, :, c]
    return A.reshape(NDW * LL, NPRED)


def _plan(offset_t, offset_n, conv_t_w, conv_n_w, W):
    """Split used features into hi (fp16) / lo (fp8) index lists."""
    A_t = _build_A(offset_t, conv_t_w, "t")
    D = _build_A(offset_n, conv_n_w, "n") - A_t
    c2 = (A_t ** 2 + D ** 2).sum(axis=1)
    used = np.where(c2 > 0)[0]
    assert len(used) <= N_HI + N_LO, f"{len(used)} used features > {N_HI + N_LO}"
    n_hi = max(N_HI, len(used) - N_LO)
    by_weight = used[np.argsort(-c2[used])]
    hi = np.sort(by_weight[:n_hi])
    lo = np.sort(by_weight[n_hi:])
    lo_mask = np.ones(N_LO)
    lo_mask[len(lo):] = 0.0
    hi_mask = np.ones(N_HI)
    hi_mask[len(hi):] = 0.0
    hi = np.concatenate([hi, np.zeros(N_HI - len(hi), np.int64)])
    lo = np.concatenate([lo, np.zeros(N_LO - len(lo), np.int64)])
    Wf = np.asarray(W, np.float64)
    fold = np.linalg.cond(Wf) < MAX_W_COND
    return hi, hi_mask, lo, lo_mask, fold, A_t, D


def _build_weights(hi, hi_mask, lo, lo_mask, fold, A_t, D, W):
    """Stationary lhsT blocks packed as one [128, NW*112] tensor.

    block 0: gate blockdiag-W [96, 112]; block 1: hi lhsT [48, 112];
    block 2: lo lhsT [128, 112]; block 3: I48; block 4 (no-fold only):
    bias-permutation identity [96, 112]. x-path cols: Pt at 12g+j,
    Pd at 64+12g+j for group-slot g in 0..3.
    """
    NW = 5 if not fold else 4
    wts = np.zeros((128, NW * 112), np.float64)
    Wf = np.asarray(W, np.float64)
    for g in range(8):
        col = 12 * (g % 4) + 64 * (g // 4)
        wts[12 * g:12 * g + 12, col:col + 12] = Wf
        if not fold:
            for q in range(12):
                wts[12 * g + q, 4 * 112 + col + q] = 1.0
    A_hi = A_t[hi] * hi_mask[:, None]
    D_hi = D[hi] * hi_mask[:, None]
    A_lo = A_t[lo] * lo_mask[:, None]
    D_lo = D[lo] * lo_mask[:, None]
    for g in range(4):
        c = 112 + 12 * g
        wts[N_HI * g:N_HI * (g + 1), c:c + 12] = A_hi
        wts[N_HI * g:N_HI * (g + 1), c + 64:c + 76] = D_hi
        c = 224 + 12 * g
        wts[N_LO * g:N_LO * (g + 1), c:c + 12] = A_lo
        wts[N_LO * g:N_LO * (g + 1), c + 64:c + 76] = D_lo
    wts[0:48, 336:384] = np.eye(48)
    return wts.astype(np.float32)


def build_program(fold=True):
    import concourse.bass as bass
    import concourse.tile as tile
    from concourse import bacc, mybir
    from concourse.bass_interp import get_hw_module

    f32 = mybir.dt.float32
    f16 = mybir.dt.float16
    e3 = mybir.dt.float8e3
    NW = 4 if fold else 5

    nc = bacc.Bacc("TRN2", target_bir_lowering=False, debug=False,
                   num_devices=N_CORES)
    xhi = nc.dram_tensor("xhi", [BPC, 2, 4 * N_HI, 512], f16,
                         kind="ExternalInput").ap()
    xlo = nc.dram_tensor("xlo", [BPC, 2, 4 * N_LO, 512], e3,
                         kind="ExternalInput").ap()
    ct = nc.dram_tensor("ct", [BPC, 96, 512], f16, kind="ExternalInput").ap()
    wts = nc.dram_tensor("wts", [128, NW * 112], f16, kind="ExternalInput").ap()
    if not fold:
        bias = nc.dram_tensor("bias", [96, 512], f16, kind="ExternalInput").ap()
    yp = nc.dram_tensor("yp", [BPC, 96, 512], f16, kind="ExternalOutput").ap()

    with tile.TileContext(nc) as tc, ExitStack() as ctx:
        consts = ctx.enter_context(tc.tile_pool(name="consts", bufs=1))
        xpool = ctx.enter_context(tc.tile_pool(name="xp", bufs=1))
        cpool = ctx.enter_context(tc.tile_pool(name="ct", bufs=1))
        spool = ctx.enter_context(tc.tile_pool(name="sig", bufs=4))
        tpool = ctx.enter_context(tc.tile_pool(name="tmp", bufs=8))
        opool = ctx.enter_context(tc.tile_pool(name="ost", bufs=1))
        xps = ctx.enter_context(
            tc.tile_pool(name="xps", bufs=4, space=bass.MemorySpace.PSUM))
        gps = xps

        # ---- all input DMAs up-front, in pipeline order ----
        w_sb = consts.tile([128, NW * 112], f16)
        nc.sync.dma_start(w_sb[:], wts[:])
        if not fold:
            bias_sb = consts.tile([96, 512], f16)
            nc.sync.dma_start(bias_sb[:], bias[:])

        ct_tiles = [cpool.tile([96, 2048], f16, name=f"ct{i}") for i in range(4)]
        xhi_tiles = [None] * 8
        xlo_tiles = [None] * 8

        def load_ct(i4, half=None):
            # half granularity for the first tile (faster pipeline fill)
            b0 = i4 * 4 if half is None else i4 * 4 + half * 2
            nb = 4 if half is None else 2
            c0 = 0 if half in (None, 0) else 1024
            nc.sync.dma_start(
                ct_tiles[i4][:, c0:c0 + nb * 512]
                .rearrange("p (b f) -> p b f", b=nb),
                ct[b0:b0 + nb].rearrange("b p f -> p b f"))

        def load_x(pair0, npair, split_batches=False):
            b0 = pair0 * 2
            thi = xpool.tile([4 * N_HI, npair * 2048], f16, name=f"xh{pair0}")
            tlo = xpool.tile([4 * N_LO, npair * 2048], e3, name=f"xl{pair0}")
            if split_batches:
                for b in range(2 * npair):
                    nc.sync.dma_start(
                        thi[:, b * 1024:(b + 1) * 1024]
                        .rearrange("p (h f) -> p h f", h=2),
                        xhi[b0 + b])
                    nc.sync.dma_start(
                        tlo[:, b * 1024:(b + 1) * 1024]
                        .rearrange("p (h f) -> p h f", h=2),
                        xlo[b0 + b])
            else:
                nc.sync.dma_start(
                    thi[:].rearrange("p (b h f) -> p b h f", b=2 * npair, h=2),
                    xhi[b0:b0 + 2 * npair].rearrange("b h p f -> p b h f"))
                nc.sync.dma_start(
                    tlo[:].rearrange("p (b h f) -> p b h f", b=2 * npair, h=2),
                    xlo[b0:b0 + 2 * npair].rearrange("b h p f -> p b h f"))
            for k in range(npair):
                xhi_tiles[pair0 + k] = (thi, k * 2048)
                xlo_tiles[pair0 + k] = (tlo, k * 2048)

        load_x(0, 1)
        load_ct(0, 0)
        load_x(1, 1)
        load_ct(0, 1)
        for i4 in range(1, 4):
            load_ct(i4)
            load_x(i4 * 2, 2)

        def w_blk(k, rows, ncols=112):
            return w_sb[rows, k * 112:k * 112 + ncols]

        # warm-up matmuls: keep PE busy through the DMA fill so the p-state
        # ramp completes before the first real matmul
        wp = xps.tile([112, 1024], f32, name="warm", tag="px")
        for i in range(N_WARM_MM):
            nc.tensor.matmul(wp[0:16, 0:WARM_FREE], w_sb[0:16, 0:16],
                             w_sb[0:16, 0:WARM_FREE], start=True, stop=True)


        # ---- per-pair stages; gates run one pair ahead ----
        def gates_stage(pair):
            i4, half = divmod(pair, 2)
            g_ps = gps.tile([112, 1024], f32, name=f"g{pair}", tag="px")
            for b in range(2):
                cs = 1024 * half + 512 * b
                nc.tensor.matmul(g_ps[:, 512 * b:512 * b + 512],
                                 w_blk(0, slice(0, 96)),
                                 ct_tiles[i4][:, cs:cs + 512],
                                 start=True, stop=fold)
            if not fold:
                for b in range(2):
                    nc.tensor.matmul(g_ps[:, 512 * b:512 * b + 512],
                                     w_blk(4, slice(0, 96)), bias_sb[:],
                                     start=False, stop=True)
            s_sb = spool.tile([112, 1024], f16)
            nc.scalar.activation(s_sb[:], g_ps[:],
                                 mybir.ActivationFunctionType.Sigmoid)
            return s_sb

        def xpath_half(pair, px, h):
            thi, hoff = xhi_tiles[pair]
            tlo, loff = xlo_tiles[pair]
            for b in range(2):
                xc = (2 * b + h) * 512
                out = px[h][:, 512 * b:512 * b + 512]
                nc.tensor.matmul(out, w_blk(1, slice(0, 4 * N_HI)),
                                 thi[:, hoff + xc:hoff + xc + 512],
                                 start=True, stop=False)
                nc.tensor.matmul(out, w_blk(2, slice(0, 4 * N_LO)),
                                 tlo[:, loff + xc:loff + xc + 512],
                                 start=False, stop=True)

        def combine_stage(pair, px, s_sb, o_sb, ocol):
            # quarter D (batch 1, h1) first: its ACT copy only needs the
            # x-path, and the GPSIMD chain behind it is the longest
            d_sb = tpool.tile([112, 512], f16, name="d")
            nc.scalar.activation(d_sb[:], px[1][:, 512:1024],
                                 mybir.ActivationFunctionType.Copy)
            td = tpool.tile([48, 512], f16, name="td")
            nc.gpsimd.tensor_mul(td[:], d_sb[64:112, :],
                                 s_sb[64:112, 512:1024])
            nc.gpsimd.tensor_add(o_sb[64:112, ocol + 512:ocol + 1024],
                                 d_sb[0:48, :], td[:])
            # half h0: merged DVE mult, PE identity-adds, merged ACT copy
            t0 = tpool.tile([48, 1024], f16, name="t0")
            nc.vector.tensor_mul(t0[:], px[0][64:112, :], s_sb[0:48, :])
            for b in range(2):
                nc.tensor.matmul(px[0][0:48, 512 * b:512 * b + 512],
                                 w_blk(3, slice(0, 48), 48),
                                 t0[:, 512 * b:512 * b + 512],
                                 start=False, stop=True, skip_group_check=True)
            nc.scalar.activation(o_sb[0:48, ocol:ocol + 1024], px[0][0:48, :],
                                 mybir.ActivationFunctionType.Copy)
            # quarter C (batch 0, h1): fully on DVE
            t1 = tpool.tile([48, 512], f16, name="t1")
            nc.vector.tensor_mul(t1[:], px[1][64:112, 0:512],
                                 s_sb[64:112, 0:512])
            nc.vector.tensor_add(o_sb[64:112, ocol:ocol + 512],
                                 px[1][0:48, 0:512], t1[:])

        # output tiles: pairs (0,1), (2,3), (4,5) share [112,2048] tiles;
        # pairs 6 and 7 get their own [112,1024] (shorter tail)
        o_tiles = []
        for k in range(3):
            o_tiles.append((opool.tile([112, 2048], f16, name=f"o{k}"),
                            [2 * k, 2 * k + 1]))
        o_tiles.append((opool.tile([112, 1024], f16, name="o6"), [6]))
        o_tiles.append((opool.tile([112, 1024], f16, name="o7"), [7]))
        pair_otile = {}
        for o_sb, pairs in o_tiles:
            for i, p in enumerate(pairs):
                pair_otile[p] = (o_sb, i * 1024, p == pairs[-1])

        def store(o_sb, pairs):
            b0, nb = 2 * pairs[0], 2 * len(pairs)
            for r0, y0 in ((0, 0), (64, 48)):
                nc.sync.dma_start(
                    yp[b0:b0 + nb, y0:y0 + 48].rearrange("b p f -> p b f"),
                    o_sb[r0:r0 + 48, :].rearrange("p (b f) -> p b f", b=nb))

        s_store = {}
        for p in range(9):
            px = None
            if p >= 1:
                px = [xps.tile([112, 1024], f32, name=f"px{p-1}_{h}", tag="px")
                      for h in range(2)]
                xpath_half(p - 1, px, 0)
                xpath_half(p - 1, px, 1)
            if p < 8:
                s_store[p] = gates_stage(p)
            if p >= 1:
                o_sb, ocol, last = pair_otile[p - 1]
                combine_stage(p - 1, px, s_store.pop(p - 1), o_sb, ocol)
                if last:
                    store(*[ot for ot in o_tiles if p - 1 in ot[1]][0])

    nc.compile()
    nc.m = get_hw_module(nc.m)
    return nc


_PROGRAMS = {}


def _get_program(fold):
    if fold not in _PROGRAMS:
        _PROGRAMS[fold] = build_program(fold)
    return _PROGRAMS[fold]


def pack_inputs(inp, ctrl, bparam, W, hi, lo, fold):
    """Host-side shard + layout packing. Returns in_maps (list of 8 dicts)."""
    import ml_dtypes
    X = np.asarray(inp, np.float32).reshape(B, 2, 4, 512, NDW * LL)
    # [B, h, g', f, k] -> [B, h, g'*nk + k, f]
    Xhi = np.ascontiguousarray(
        X[..., hi].transpose(0, 1, 2, 4, 3)).reshape(B, 2, 4 * N_HI, 512)
    Xlo = np.ascontiguousarray(
        X[..., lo].transpose(0, 1, 2, 4, 3)).reshape(B, 2, 4 * N_LO, 512)
    ctf = np.asarray(ctrl, np.float64)
    if fold:
        binv = np.asarray(bparam, np.float64) @ np.linalg.inv(
            np.asarray(W, np.float64))
        ctf = ctf + binv[None, :, :]
    CT = np.ascontiguousarray(
        ctf.astype(np.float32).reshape(B, NG, 512, 12).transpose(0, 1, 3, 2)
    ).reshape(B, 96, 512)
    Xhi = Xhi.astype(np.float16)
    Xlo = Xlo.astype(ml_dtypes.float8_e3m4)
    CT = CT.astype(np.float16)
    in_maps = []
    for c in range(N_CORES):
        sl = slice(c * BPC, (c + 1) * BPC)
        in_maps.append({"xhi": Xhi[sl], "xlo": Xlo[sl], "ct": CT[sl]})
    return in_maps


def unpack_output(results):
    yp = np.concatenate([r["yp"].astype(np.float32) for r in results], axis=0)
    return np.ascontiguousarray(
        yp.reshape(B, NG, 12, 512).transpose(0, 1, 3, 2)
    ).reshape(B, NNODES, NPRED)


def kernel(inp, ctrl, offset_t, offset_n, conv_t_w, conv_t_b, conv_n_w,
           conv_n_b, W, bparam):
    from concourse.bass_utils import run_bass_kernel_spmd

    hi, hi_mask, lo, lo_mask, fold, A_t, D = _plan(
        offset_t, offset_n, conv_t_w, conv_n_w, W)
    nc = _get_program(fold)
    wts_np = _build_weights(hi, hi_mask, lo, lo_mask, fold, A_t, D, W)
    in_maps = pack_inputs(inp, ctrl, bparam, W, hi, lo, fold)
    for m in in_maps:
        m["wts"] = wts_np.astype(np.float16)
        if not fold:
            bias_t = np.ascontiguousarray(
                np.asarray(bparam, np.float32).reshape(NG, 512, 12)
                .transpose(0, 2, 1)).reshape(96, 512)
            m["bias"] = bias_t.astype(np.float16)
    res = run_bass_kernel_spmd(nc, in_maps, core_ids=list(range(N_CORES)))
    out = unpack_output(res.results)
    # Conv biases are zeros in this module's init, so the device kernel omits
    # them. If ever nonzero, apply the exact correction on the host.
    ctb = float(np.asarray(conv_t_b).reshape(-1)[0])
    cnb = np.asarray(conv_n_b, np.float32)
    if ctb != 0.0 or np.any(cnb != 0.0):
        G = np.asarray(ctrl, np.float32).reshape(B * NNODES, NPRED) @ np.asarray(
            W, np.float32)
        G += np.tile(np.asarray(bparam, np.float32), (B, 1))
        S = 1.0 / (1.0 + np.exp(-G))
        out = out + (ctb + (cnb[None, :] - ctb) * S).reshape(B, NNODES, NPRED)
    return out.astype(np.float32)


# revision 32
# speedup vs baseline: 1.0163x; 1.0163x over previous
"""Trainium2 Bass kernel for nn_DeformableConvStandard.

The deformable interpolation + both convs are linear in `inp` once the
(tiny) offsets are known, so the whole module collapses to

    out = Pt + Pd * sigmoid(ctrl' @ W),   Pt = X @ A_t,  Pd = X @ D

with A_t, D: [48, 12] host-built from offsets/conv weights, and the gate
bias pre-folded into ctrl' = ctrl + bparam @ W^-1 on the host. Rows of
[A_t|D] that are identically zero (deform positions never sampled) are
pruned from the streamed X features.

Per-core layout (16 batches = 8 pairs of 2): each pair's x-path runs in
two 2-bank PSUM tiles [112, 1024] (cols = batch), one per group-half h,
with Pt at partitions 0-47 and Pd at 64-111 (48-part blocks may only
start at 0/64). The streamed features split by weight sensitivity into
12 fp16 ("hi") + 32 fp8-e3m4 ("lo") contraction chunks, which cuts the
dominant DMA stream by 30% at ~0.9e-2 rel err. The combine
out = Pt + Pd*S is spread over all four compute engines per pair:
  - one merged DVE mult [48,1024] for half h0, two PE identity-add
    matmuls, one merged ACT copy [48,1024] to the fp16 out tile;
  - quarter C (batch0, h1) fully on DVE ([48,512] mult+add);
  - quarter D (batch1, h1): ACT copies the PSUM tile to SBUF, then the
    (otherwise idle) GPSIMD engine does mult+add SBUF-only.
One merged sigmoid [112,1024] per pair (gate PSUM tile spans 2 banks).
All input DMAs are hoisted up-front on the sync queue; output stores
follow on the same queue (2-pair output tiles, last 2 pairs split for a
shorter tail).
"""
import numpy as np
from contextlib import ExitStack

N_CORES = 8
B, NNODES = 128, 4096
NDW, LL, NPRED, NDRIFT = 3, 16, 12, 2
BPC = B // N_CORES          # batches per core: 16
NG = NNODES // 512          # node groups per batch: 8
N_HI = 12                   # fp16-streamed features (4*N_HI <= 128)
N_LO = 32                   # fp8e3m4-streamed features (4*N_LO <= 128)
MAX_W_COND = 1e4            # fold bias into ctrl only if W is this well-posed
N_WARM_MM = 8              # dummy PE warm-up matmuls (p-state ramp + fill)
WARM_FREE = 400             # free size of each warm-up matmul


def _build_A(offset, conv_w, mode):
    """A [48, 12] with pred = X @ A for X [rows, 48], feature = d*16+l."""
    off = np.asarray(offset, np.float32)
    pos = np.tanh(off) * np.float32(NDRIFT) + (
        np.arange(NPRED, dtype=np.float32) + np.float32(NDRIFT)
    )[None, :]
    key = np.floor(pos)
    frac = (pos - key).astype(np.float64)
    idx = key.astype(np.int32)
    M = np.zeros((NDW, LL, NPRED), np.float64)
    for d in range(NDW):
        for j in range(NPRED):
            M[d, idx[d, j], j] += 1.0 - frac[d, j]
            M[d, idx[d, j] + 1, j] += frac[d, j]
    A = np.zeros((NDW, LL, NPRED), np.float64)
    w = np.asarray(conv_w, np.float64)
    if mode == "t":
        for p in range(NPRED):
            for k in range(3):
                j = p + k - 1
                if 0 <= j < NPRED:
                    A[:, :, p] += w[0, :, k][:, None] * M[:, :, j]
    else:
        for o in range(NPRED):
            for d in range(NDW):
                for c in range(NPRED):
                    A[d, :, o] += w[o, c, d] * M[d, :, c]
    return A.reshape(NDW * LL, NPRED)


def _plan(offset_t, offset_n, conv_t_w, conv_n_w, W):
    """Split used features into hi (fp16) / lo (fp8) index lists."""
    A_t = _build_A(offset_t, conv_t_w, "t")
    D = _build_A(offset_n, conv_n_w, "n") - A_t
    c2 = (A_t ** 2 + D ** 2).sum(axis=1)
    used = np.where(c2 > 0)[0]
    assert len(used) <= N_HI + N_LO, f"{len(used)} used features > {N_HI + N_LO}"
    n_hi = max(N_HI, len(used) - N_LO)
    by_weight = used[np.argsort(-c2[used])]
    hi = np.sort(by_weight[:n_hi])
    lo = np.sort(by_weight[n_hi:])
    lo_mask = np.ones(N_LO)
    lo_mask[len(lo):] = 0.0
    hi_mask = np.ones(N_HI)
    hi_mask[len(hi):] = 0.0
    hi = np.concatenate([hi, np.zeros(N_HI - len(hi), np.int64)])
    lo = np.concatenate([lo, np.zeros(N_LO - len(lo), np.int64)])
    Wf = np.asarray(W, np.float64)
    fold = np.linalg.cond(Wf) < MAX_W_COND
    return hi, hi_mask, lo, lo_mask, fold, A_t, D


def _build_weights(hi, hi_mask, lo, lo_mask, fold, A_t, D, W):
    """Stationary lhsT blocks packed as one [128, NW*112] tensor.

    block 0: gate blockdiag-W [96, 112]; block 1: hi lhsT [48, 112];
    block 2: lo lhsT [128, 112]; block 3: I48; block 4 (no-fold only):
    bias-permutation identity [96, 112]. x-path cols: Pt at 12g+j,
    Pd at 64+12g+j for group-slot g in 0..3.
    """
    NW = 5 if not fold else 4
    wts = np.zeros((128, NW * 112), np.float64)
    Wf = np.asarray(W, np.float64)
    for g in range(8):
        col = 12 * (g % 4) + 64 * (g // 4)
        wts[12 * g:12 * g + 12, col:col + 12] = Wf
        if not fold:
            for q in range(12):
                wts[12 * g + q, 4 * 112 + col + q] = 1.0
    A_hi = A_t[hi] * hi_mask[:, None]
    D_hi = D[hi] * hi_mask[:, None]
    A_lo = A_t[lo] * lo_mask[:, None]
    D_lo = D[lo] * lo_mask[:, None]
    for g in range(4):
        c = 112 + 12 * g
        wts[N_HI * g:N_HI * (g + 1), c:c + 12] = A_hi
        wts[N_HI * g:N_HI * (g + 1), c + 64:c + 76] = D_hi
        c = 224 + 12 * g
        wts[N_LO * g:N_LO * (g + 1), c:c + 12] = A_lo
        wts[N_LO * g:N_LO * (g + 1), c + 64:c + 76] = D_lo
    wts[0:48, 336:384] = np.eye(48)
    return wts.astype(np.float32)


def build_program(fold=True):
    import concourse.bass as bass
    import concourse.tile as tile
    from concourse import bacc, mybir
    from concourse.bass_interp import get_hw_module

    f32 = mybir.dt.float32
    f16 = mybir.dt.float16
    e3 = mybir.dt.float8e3
    NW = 4 if fold else 5

    nc = bacc.Bacc("TRN2", target_bir_lowering=False, debug=False,
                   num_devices=N_CORES)
    xhi = nc.dram_tensor("xhi", [BPC, 2, 4 * N_HI, 512], f16,
                         kind="ExternalInput").ap()
    xlo = nc.dram_tensor("xlo", [BPC, 2, 4 * N_LO, 512], e3,
                         kind="ExternalInput").ap()
    ct = nc.dram_tensor("ct", [BPC, 96, 512], f16, kind="ExternalInput").ap()
    wts = nc.dram_tensor("wts", [128, NW * 112], f16, kind="ExternalInput").ap()
    if not fold:
        bias = nc.dram_tensor("bias", [96, 512], f16, kind="ExternalInput").ap()
    yp = nc.dram_tensor("yp", [BPC, 96, 512], f16, kind="ExternalOutput").ap()

    with tile.TileContext(nc) as tc, ExitStack() as ctx:
        consts = ctx.enter_context(tc.tile_pool(name="consts", bufs=1))
        xpool = ctx.enter_context(tc.tile_pool(name="xp", bufs=1))
        cpool = ctx.enter_context(tc.tile_pool(name="ct", bufs=1))
        spool = ctx.enter_context(tc.tile_pool(name="sig", bufs=4))
        tpool = ctx.enter_context(tc.tile_pool(name="tmp", bufs=8))
        opool = ctx.enter_context(tc.tile_pool(name="ost", bufs=1))
        xps = ctx.enter_context(
            tc.tile_pool(name="xps", bufs=4, space=bass.MemorySpace.PSUM))
        gps = xps

        # ---- all input DMAs up-front, in pipeline order ----
        w_sb = consts.tile([128, NW * 112], f16)
        nc.sync.dma_start(w_sb[:], wts[:])
        if not fold:
            bias_sb = consts.tile([96, 512], f16)
            nc.sync.dma_start(bias_sb[:], bias[:])

        ct_tiles = [cpool.tile([96, 2048], f16, name=f"ct{i}") for i in range(4)]
        xhi_tiles = [None] * 8
        xlo_tiles = [None] * 8

        def load_ct(i4, half=None):
            # half granularity for the first tile (faster pipeline fill)
            b0 = i4 * 4 if half is None else i4 * 4 + half * 2
            nb = 4 if half is None else 2
            c0 = 0 if half in (None, 0) else 1024
            nc.sync.dma_start(
                ct_tiles[i4][:, c0:c0 + nb * 512]
                .rearrange("p (b f) -> p b f", b=nb),
                ct[b0:b0 + nb].rearrange("b p f -> p b f"))

        def load_x(pair0, npair, split_batches=False):
            b0 = pair0 * 2
            thi = xpool.tile([4 * N_HI, npair * 2048], f16, name=f"xh{pair0}")
            tlo = xpool.tile([4 * N_LO, npair * 2048], e3, name=f"xl{pair0}")
            if split_batches:
                for b in range(2 * npair):
                    nc.sync.dma_start(
                        thi[:, b * 1024:(b + 1) * 1024]
                        .rearrange("p (h f) -> p h f", h=2),
                        xhi[b0 + b])
                    nc.sync.dma_start(
                        tlo[:, b * 1024:(b + 1) * 1024]
                        .rearrange("p (h f) -> p h f", h=2),
                        xlo[b0 + b])
            else:
                nc.sync.dma_start(
                    thi[:].rearrange("p (b h f) -> p b h f", b=2 * npair, h=2),
                    xhi[b0:b0 + 2 * npair].rearrange("b h p f -> p b h f"))
                nc.sync.dma_start(
                    tlo[:].rearrange("p (b h f) -> p b h f", b=2 * npair, h=2),
                    xlo[b0:b0 + 2 * npair].rearrange("b h p f -> p b h f"))
            for k in range(npair):
                xhi_tiles[pair0 + k] = (thi, k * 2048)
                xlo_tiles[pair0 + k] = (tlo, k * 2048)

        load_x(0, 1)
        load_ct(0, 0)
        load_x(1, 1)
        load_ct(0, 1)
        for i4 in range(1, 4):
            load_ct(i4)
            load_x(i4 * 2, 2)

        def w_blk(k, rows, ncols=112):
            return w_sb[rows, k * 112:k * 112 + ncols]

        # warm-up matmuls: keep PE busy through the DMA fill so the p-state
        # ramp completes before the first real matmul
        wp = xps.tile([112, 1024], f32, name="warm", tag="px")
        for i in range(N_WARM_MM):
            nc.tensor.matmul(wp[0:16, 0:WARM_FREE], w_sb[0:16, 0:16],
                             w_sb[0:16, 0:WARM_FREE], start=True, stop=True)


        # ---- per-pair stages; gates run one pair ahead ----
        def gates_stage(pair):
            i4, half = divmod(pair, 2)
            g_ps = gps.tile([112, 1024], f32, name=f"g{pair}", tag="px")
            for b in range(2):
                cs = 1024 * half + 512 * b
                nc.tensor.matmul(g_ps[:, 512 * b:512 * b + 512],
                                 w_blk(0, slice(0, 96)),
                                 ct_tiles[i4][:, cs:cs + 512],
                                 start=True, stop=fold)
            if not fold:
                for b in range(2):
                    nc.tensor.matmul(g_ps[:, 512 * b:512 * b + 512],
                                     w_blk(4, slice(0, 96)), bias_sb[:],
                                     start=False, stop=True)
            s_sb = spool.tile([112, 1024], f16)
            nc.scalar.activation(s_sb[:], g_ps[:],
                                 mybir.ActivationFunctionType.Sigmoid)
            return s_sb

        def xpath_half(pair, px, h):
            thi, hoff = xhi_tiles[pair]
            tlo, loff = xlo_tiles[pair]
            for b in range(2):
                xc = (2 * b + h) * 512
                out = px[h][:, 512 * b:512 * b + 512]
                nc.tensor.matmul(out, w_blk(1, slice(0, 4 * N_HI)),
                                 thi[:, hoff + xc:hoff + xc + 512],
                                 start=True, stop=False)
                nc.tensor.matmul(out, w_blk(2, slice(0, 4 * N_LO)),
                                 tlo[:, loff + xc:loff + xc + 512],
                                 start=False, stop=True)

        def combine_d(pair, px, s_sb, o_sb, ocol):
            # quarter D (batch 1, h1) first: its ACT copy only needs the
            # x-path, and the GPSIMD chain behind it is the longest
            d_sb = tpool.tile([112, 512], f16, name="d")
            nc.scalar.activation(d_sb[:], px[1][:, 512:1024],
                                 mybir.ActivationFunctionType.Copy)
            td = tpool.tile([48, 512], f16, name="td")
            nc.gpsimd.tensor_mul(td[:], d_sb[64:112, :],
                                 s_sb[64:112, 512:1024])
            nc.gpsimd.tensor_add(o_sb[64:112, ocol + 512:ocol + 1024],
                                 d_sb[0:48, :], td[:])

        def combine_rest(pair, px, s_sb, o_sb, ocol):
            # half h0: merged DVE mult, PE identity-adds, merged ACT copy
            t0 = tpool.tile([48, 1024], f16, name="t0")
            nc.vector.tensor_mul(t0[:], px[0][64:112, :], s_sb[0:48, :])
            for b in range(2):
                nc.tensor.matmul(px[0][0:48, 512 * b:512 * b + 512],
                                 w_blk(3, slice(0, 48), 48),
                                 t0[:, 512 * b:512 * b + 512],
                                 start=False, stop=True, skip_group_check=True)
            nc.scalar.activation(o_sb[0:48, ocol:ocol + 1024], px[0][0:48, :],
                                 mybir.ActivationFunctionType.Copy)
            # quarter C (batch 0, h1): fully on DVE
            t1 = tpool.tile([48, 512], f16, name="t1")
            nc.vector.tensor_mul(t1[:], px[1][64:112, 0:512],
                                 s_sb[64:112, 0:512])
            nc.vector.tensor_add(o_sb[64:112, ocol:ocol + 512],
                                 px[1][0:48, 0:512], t1[:])

        # output tiles: pairs (0,1), (2,3), (4,5) share [112,2048] tiles;
        # pairs 6 and 7 get their own [112,1024] (shorter tail)
        o_tiles = []
        for k in range(3):
            o_tiles.append((opool.tile([112, 2048], f16, name=f"o{k}"),
                            [2 * k, 2 * k + 1]))
        o_tiles.append((opool.tile([112, 1024], f16, name="o6"), [6]))
        o_tiles.append((opool.tile([112, 1024], f16, name="o7"), [7]))
        pair_otile = {}
        for o_sb, pairs in o_tiles:
            for i, p in enumerate(pairs):
                pair_otile[p] = (o_sb, i * 1024, p == pairs[-1])

        def store(o_sb, pairs):
            b0, nb = 2 * pairs[0], 2 * len(pairs)
            for r0, y0 in ((0, 0), (64, 48)):
                nc.sync.dma_start(
                    yp[b0:b0 + nb, y0:y0 + 48].rearrange("b p f -> p b f"),
                    o_sb[r0:r0 + 48, :].rearrange("p (b f) -> p b f", b=nb))

        s_store = {}
        for p in range(9):
            px = None
            if p >= 1:
                # allocation order [px0, gate, px1] makes the pool-slot reuse
                # edges land on early-freed tiles: px1(p) <- gate(p) (freed by
                # the sigmoid), gate(p+1) <- px0(p) (copy_h0, but gates have a
                # pair of lead slack), px0(p) <- px1(p-1) (freed mid-combine)
                px = [xps.tile([112, 1024], f32, name=f"px{p-1}_0", tag="px"),
                      None]
                xpath_half(p - 1, px, 0)
            if p < 8:
                s_store[p] = gates_stage(p)
            if p >= 1:
                px[1] = xps.tile([112, 1024], f32, name=f"px{p-1}_1", tag="px")
                xpath_half(p - 1, px, 1)
                o_sb, ocol, last = pair_otile[p - 1]
                s_prev = s_store.pop(p - 1)
                combine_d(p - 1, px, s_prev, o_sb, ocol)
                combine_rest(p - 1, px, s_prev, o_sb, ocol)
                if last:
                    store(*[ot for ot in o_tiles if p - 1 in ot[1]][0])

    nc.compile()
    nc.m = get_hw_module(nc.m)
    return nc


_PROGRAMS = {}


def _get_program(fold):
    if fold not in _PROGRAMS:
        _PROGRAMS[fold] = build_program(fold)
    return _PROGRAMS[fold]


def pack_inputs(inp, ctrl, bparam, W, hi, lo, fold):
    """Host-side shard + layout packing. Returns in_maps (list of 8 dicts)."""
    import ml_dtypes
    X = np.asarray(inp, np.float32).reshape(B, 2, 4, 512, NDW * LL)
    # [B, h, g', f, k] -> [B, h, g'*nk + k, f]
    Xhi = np.ascontiguousarray(
        X[..., hi].transpose(0, 1, 2, 4, 3)).reshape(B, 2, 4 * N_HI, 512)
    Xlo = np.ascontiguousarray(
        X[..., lo].transpose(0, 1, 2, 4, 3)).reshape(B, 2, 4 * N_LO, 512)
    ctf = np.asarray(ctrl, np.float64)
    if fold:
        binv = np.asarray(bparam, np.float64) @ np.linalg.inv(
            np.asarray(W, np.float64))
        ctf = ctf + binv[None, :, :]
    CT = np.ascontiguousarray(
        ctf.astype(np.float32).reshape(B, NG, 512, 12).transpose(0, 1, 3, 2)
    ).reshape(B, 96, 512)
    Xhi = Xhi.astype(np.float16)
    Xlo = Xlo.astype(ml_dtypes.float8_e3m4)
    CT = CT.astype(np.float16)
    in_maps = []
    for c in range(N_CORES):
        sl = slice(c * BPC, (c + 1) * BPC)
        in_maps.append({"xhi": Xhi[sl], "xlo": Xlo[sl], "ct": CT[sl]})
    return in_maps


def unpack_output(results):
    yp = np.concatenate([r["yp"].astype(np.float32) for r in results], axis=0)
    return np.ascontiguousarray(
        yp.reshape(B, NG, 12, 512).transpose(0, 1, 3, 2)
    ).reshape(B, NNODES, NPRED)


def kernel(inp, ctrl, offset_t, offset_n, conv_t_w, conv_t_b, conv_n_w,
           conv_n_b, W, bparam):
    from concourse.bass_utils import run_bass_kernel_spmd

    hi, hi_mask, lo, lo_mask, fold, A_t, D = _plan(
        offset_t, offset_n, conv_t_w, conv_n_w, W)
    nc = _get_program(fold)
    wts_np = _build_weights(hi, hi_mask, lo, lo_mask, fold, A_t, D, W)
    in_maps = pack_inputs(inp, ctrl, bparam, W, hi, lo, fold)
    for m in in_maps:
        m["wts"] = wts_np.astype(np.float16)
        if not fold:
            bias_t = np.ascontiguousarray(
                np.asarray(bparam, np.float32).reshape(NG, 512, 12)
                .transpose(0, 2, 1)).reshape(96, 512)
            m["bias"] = bias_t.astype(np.float16)
    res = run_bass_kernel_spmd(nc, in_maps, core_ids=list(range(N_CORES)))
    out = unpack_output(res.results)
    # Conv biases are zeros in this module's init, so the device kernel omits
    # them. If ever nonzero, apply the exact correction on the host.
    ctb = float(np.asarray(conv_t_b).reshape(-1)[0])
    cnb = np.asarray(conv_n_b, np.float32)
    if ctb != 0.0 or np.any(cnb != 0.0):
        G = np.asarray(ctrl, np.float32).reshape(B * NNODES, NPRED) @ np.asarray(
            W, np.float32)
        G += np.tile(np.asarray(bparam, np.float32), (B, 1))
        S = 1.0 / (1.0 + np.exp(-G))
        out = out + (ctb + (cnb[None, :] - ctb) * S).reshape(B, NNODES, NPRED)
    return out.astype(np.float32)


# revision 43
# speedup vs baseline: 1.0267x; 1.0103x over previous
"""Trainium2 Bass kernel for nn_DeformableConvStandard.

The deformable interpolation + both convs are linear in `inp` once the
(tiny) offsets are known, so the whole module collapses to

    out = Pt + Pd * sigmoid(ctrl' @ W),   Pt = X @ A_t,  Pd = X @ D

with A_t, D: [48, 12] host-built from offsets/conv weights, and the gate
bias pre-folded into ctrl' = ctrl + bparam @ W^-1 on the host. Rows of
[A_t|D] that are identically zero (deform positions never sampled) are
pruned from the streamed X features.

Per-core layout (16 batches = 8 pairs of 2): each pair's x-path runs in
two 2-bank PSUM tiles [112, 1024] (cols = batch), one per group-half h,
with Pt at partitions 0-47 and Pd at 64-111 (48-part blocks may only
start at 0/64). The streamed features split by weight sensitivity into
12 fp16 ("hi") + 32 fp8-e3m4 ("lo") contraction chunks, which cuts the
dominant DMA stream by 30% at ~0.9e-2 rel err. The combine
out = Pt + Pd*S is spread over all four compute engines per pair:
  - one merged DVE mult [48,1024] for half h0, two PE identity-add
    matmuls, one merged ACT copy [48,1024] to the fp16 out tile;
  - quarter C (batch0, h1) fully on DVE ([48,512] mult+add);
  - quarter D (batch1, h1): ACT copies the PSUM tile to SBUF, then the
    (otherwise idle) GPSIMD engine does mult+add SBUF-only.
One merged sigmoid [112,1024] per pair (gate PSUM tile spans 2 banks).
All input DMAs are hoisted up-front on the sync queue; output stores
follow on the same queue (2-pair output tiles, last 2 pairs split for a
shorter tail).
"""
import numpy as np
from contextlib import ExitStack

N_CORES = 8
B, NNODES = 128, 4096
NDW, LL, NPRED, NDRIFT = 3, 16, 12, 2
BPC = B // N_CORES          # batches per core: 16
NG = NNODES // 512          # node groups per batch: 8
N_HI = 12                   # fp16-streamed features (4*N_HI <= 128)
N_LO = 32                   # fp8e3m4-streamed features (4*N_LO <= 128)
MAX_W_COND = 1e4            # fold bias into ctrl only if W is this well-posed
N_WARM_MM = 8              # dummy PE warm-up matmuls (p-state ramp + fill)
WARM_FREE = 400             # free size of each warm-up matmul


def _build_A(offset, conv_w, mode):
    """A [48, 12] with pred = X @ A for X [rows, 48], feature = d*16+l."""
    off = np.asarray(offset, np.float32)
    pos = np.tanh(off) * np.float32(NDRIFT) + (
        np.arange(NPRED, dtype=np.float32) + np.float32(NDRIFT)
    )[None, :]
    key = np.floor(pos)
    frac = (pos - key).astype(np.float64)
    idx = key.astype(np.int32)
    M = np.zeros((NDW, LL, NPRED), np.float64)
    for d in range(NDW):
        for j in range(NPRED):
            M[d, idx[d, j], j] += 1.0 - frac[d, j]
            M[d, idx[d, j] + 1, j] += frac[d, j]
    A = np.zeros((NDW, LL, NPRED), np.float64)
    w = np.asarray(conv_w, np.float64)
    if mode == "t":
        for p in range(NPRED):
            for k in range(3):
                j = p + k - 1
                if 0 <= j < NPRED:
                    A[:, :, p] += w[0, :, k][:, None] * M[:, :, j]
    else:
        for o in range(NPRED):
            for d in range(NDW):
                for c in range(NPRED):
                    A[d, :, o] += w[o, c, d] * M[d, :, c]
    return A.reshape(NDW * LL, NPRED)


def _plan(offset_t, offset_n, conv_t_w, conv_n_w, W):
    """Split used features into hi (fp16) / lo (fp8) index lists."""
    A_t = _build_A(offset_t, conv_t_w, "t")
    D = _build_A(offset_n, conv_n_w, "n") - A_t
    c2 = (A_t ** 2 + D ** 2).sum(axis=1)
    used = np.where(c2 > 0)[0]
    assert len(used) <= N_HI + N_LO, f"{len(used)} used features > {N_HI + N_LO}"
    n_hi = max(N_HI, len(used) - N_LO)
    by_weight = used[np.argsort(-c2[used])]
    hi = np.sort(by_weight[:n_hi])
    lo = np.sort(by_weight[n_hi:])
    lo_mask = np.ones(N_LO)
    lo_mask[len(lo):] = 0.0
    hi_mask = np.ones(N_HI)
    hi_mask[len(hi):] = 0.0
    hi = np.concatenate([hi, np.zeros(N_HI - len(hi), np.int64)])
    lo = np.concatenate([lo, np.zeros(N_LO - len(lo), np.int64)])
    Wf = np.asarray(W, np.float64)
    fold = np.linalg.cond(Wf) < MAX_W_COND
    return hi, hi_mask, lo, lo_mask, fold, A_t, D


def _build_weights(hi, hi_mask, lo, lo_mask, fold, A_t, D, W):
    """Stationary lhsT blocks packed as one [128, NW*112] tensor.

    block 0: gate blockdiag-W [96, 112]; block 1: hi lhsT [48, 112];
    block 2: lo lhsT [128, 112]; block 3: I48; block 4 (no-fold only):
    bias-permutation identity [96, 112]. x-path cols: Pt at 12g+j,
    Pd at 64+12g+j for group-slot g in 0..3.
    """
    NW = 5 if not fold else 4
    wts = np.zeros((128, NW * 112), np.float64)
    Wf = np.asarray(W, np.float64)
    for g in range(8):
        col = 12 * (g % 4) + 64 * (g // 4)
        wts[12 * g:12 * g + 12, col:col + 12] = Wf
        if not fold:
            for q in range(12):
                wts[12 * g + q, 4 * 112 + col + q] = 1.0
    A_hi = A_t[hi] * hi_mask[:, None]
    D_hi = D[hi] * hi_mask[:, None]
    A_lo = A_t[lo] * lo_mask[:, None]
    D_lo = D[lo] * lo_mask[:, None]
    for g in range(4):
        c = 112 + 12 * g
        wts[N_HI * g:N_HI * (g + 1), c:c + 12] = A_hi
        wts[N_HI * g:N_HI * (g + 1), c + 64:c + 76] = D_hi
        c = 224 + 12 * g
        wts[N_LO * g:N_LO * (g + 1), c:c + 12] = A_lo
        wts[N_LO * g:N_LO * (g + 1), c + 64:c + 76] = D_lo
    wts[0:48, 336:384] = np.eye(48)
    return wts.astype(np.float32)


def build_program(fold=True):
    import concourse.bass as bass
    import concourse.tile as tile
    from concourse import bacc, mybir
    from concourse.bass_interp import get_hw_module

    f32 = mybir.dt.float32
    f16 = mybir.dt.float16
    e3 = mybir.dt.float8e3
    NW = 4 if fold else 5

    nc = bacc.Bacc("TRN2", target_bir_lowering=False, debug=False,
                   num_devices=N_CORES)
    xhi = nc.dram_tensor("xhi", [BPC, 2, 4 * N_HI, 512], f16,
                         kind="ExternalInput").ap()
    xlo = nc.dram_tensor("xlo", [BPC, 2, 4 * N_LO, 512], e3,
                         kind="ExternalInput").ap()
    ct = nc.dram_tensor("ct", [BPC, 96, 512], f16, kind="ExternalInput").ap()
    wts = nc.dram_tensor("wts", [128, NW * 112], f16, kind="ExternalInput").ap()
    if not fold:
        bias = nc.dram_tensor("bias", [96, 512], f16, kind="ExternalInput").ap()
    yp = nc.dram_tensor("yp", [BPC, 96, 512], f16, kind="ExternalOutput").ap()

    with tile.TileContext(nc) as tc, ExitStack() as ctx:
        consts = ctx.enter_context(tc.tile_pool(name="consts", bufs=1))
        xpool = ctx.enter_context(tc.tile_pool(name="xp", bufs=1))
        cpool = ctx.enter_context(tc.tile_pool(name="ct", bufs=1))
        spool = ctx.enter_context(tc.tile_pool(name="sig", bufs=4))
        tpool = ctx.enter_context(tc.tile_pool(name="tmp", bufs=8))
        opool = ctx.enter_context(tc.tile_pool(name="ost", bufs=1))
        xps = ctx.enter_context(
            tc.tile_pool(name="xps", bufs=4, space=bass.MemorySpace.PSUM))
        gps = xps

        # ---- all input DMAs up-front, in pipeline order ----
        w_sb = consts.tile([128, NW * 112], f16)
        nc.sync.dma_start(w_sb[:], wts[:])
        if not fold:
            bias_sb = consts.tile([96, 512], f16)
            nc.sync.dma_start(bias_sb[:], bias[:])

        ct_tiles = [cpool.tile([96, 2048], f16, name=f"ct{i}") for i in range(4)]
        xhi_tiles = [None] * 8
        xlo_tiles = [None] * 8

        def load_ct(i4, half=None):
            # half granularity for the first tile (faster pipeline fill)
            b0 = i4 * 4 if half is None else i4 * 4 + half * 2
            nb = 4 if half is None else 2
            c0 = 0 if half in (None, 0) else 1024
            nc.sync.dma_start(
                ct_tiles[i4][:, c0:c0 + nb * 512]
                .rearrange("p (b f) -> p b f", b=nb),
                ct[b0:b0 + nb].rearrange("b p f -> p b f"))

        def load_x(pair0, npair, split_batches=False):
            b0 = pair0 * 2
            thi = xpool.tile([4 * N_HI, npair * 2048], f16, name=f"xh{pair0}")
            tlo = xpool.tile([4 * N_LO, npair * 2048], e3, name=f"xl{pair0}")
            if split_batches:
                for b in range(2 * npair):
                    nc.sync.dma_start(
                        thi[:, b * 1024:(b + 1) * 1024]
                        .rearrange("p (h f) -> p h f", h=2),
                        xhi[b0 + b])
                    nc.sync.dma_start(
                        tlo[:, b * 1024:(b + 1) * 1024]
                        .rearrange("p (h f) -> p h f", h=2),
                        xlo[b0 + b])
            else:
                nc.sync.dma_start(
                    thi[:].rearrange("p (b h f) -> p b h f", b=2 * npair, h=2),
                    xhi[b0:b0 + 2 * npair].rearrange("b h p f -> p b h f"))
                nc.sync.dma_start(
                    tlo[:].rearrange("p (b h f) -> p b h f", b=2 * npair, h=2),
                    xlo[b0:b0 + 2 * npair].rearrange("b h p f -> p b h f"))
            for k in range(npair):
                xhi_tiles[pair0 + k] = (thi, k * 2048)
                xlo_tiles[pair0 + k] = (tlo, k * 2048)

        load_x(0, 1)
        load_ct(0, 0)
        load_x(1, 1)
        load_ct(0, 1)
        for i4 in range(1, 4):
            load_ct(i4)
            load_x(i4 * 2, 2)

        def w_blk(k, rows, ncols=112):
            return w_sb[rows, k * 112:k * 112 + ncols]

        # warm-up matmuls: keep PE busy through the DMA fill so the p-state
        # ramp completes before the first real matmul
        wp = xps.tile([112, 1024], f32, name="warm", tag="px")
        for i in range(N_WARM_MM):
            nc.tensor.matmul(wp[0:16, 0:WARM_FREE], w_sb[0:16, 0:16],
                             w_sb[0:16, 0:WARM_FREE], start=True, stop=True)


        # ---- per-pair stages; gates run one pair ahead ----
        def gates_stage(pair):
            i4, half = divmod(pair, 2)
            g_ps = gps.tile([112, 1024], f32, name=f"g{pair}", tag="px")
            for b in range(2):
                cs = 1024 * half + 512 * b
                nc.tensor.matmul(g_ps[:, 512 * b:512 * b + 512],
                                 w_blk(0, slice(0, 96)),
                                 ct_tiles[i4][:, cs:cs + 512],
                                 start=True, stop=fold)
            if not fold:
                for b in range(2):
                    nc.tensor.matmul(g_ps[:, 512 * b:512 * b + 512],
                                     w_blk(4, slice(0, 96)), bias_sb[:],
                                     start=False, stop=True)
            s_sb = spool.tile([112, 1024], f16)
            nc.scalar.activation(s_sb[:], g_ps[:],
                                 mybir.ActivationFunctionType.Sigmoid)
            return s_sb

        def xpath_half(pair, px, h):
            thi, hoff = xhi_tiles[pair]
            tlo, loff = xlo_tiles[pair]
            for b in range(2):
                xc = (2 * b + h) * 512
                out = px[h][:, 512 * b:512 * b + 512]
                nc.tensor.matmul(out, w_blk(1, slice(0, 4 * N_HI)),
                                 thi[:, hoff + xc:hoff + xc + 512],
                                 start=True, stop=False)
                nc.tensor.matmul(out, w_blk(2, slice(0, 4 * N_LO)),
                                 tlo[:, loff + xc:loff + xc + 512],
                                 start=False, stop=True)

        def combine_d(pair, px, s_sb, o_sb, ocol):
            # quarter D (h1 cols cs:1024) first: its ACT copy only needs
            # the x-path, and the GPSIMD chain behind it is the longest
            cs = CSPL_TAIL if pair == 7 else CSPL
            dw = 1024 - cs
            d_sb = tpool.tile([112, dw], f16, name="d")
            nc.scalar.activation(d_sb[:], px[1][:, cs:1024],
                                 mybir.ActivationFunctionType.Copy)
            td = tpool.tile([48, dw], f16, name="td")
            nc.gpsimd.tensor_mul(td[:], d_sb[64:112, :],
                                 s_sb[64:112, cs:1024])
            nc.gpsimd.tensor_add(o_sb[64:112, ocol + cs:ocol + 1024],
                                 d_sb[0:48, :], td[:])

        def combine_rest(pair, px, s_sb, o_sb, ocol):
            # half h0: merged DVE mult, PE identity-adds, merged ACT copy
            t0 = tpool.tile([48, 1024], f16, name="t0")
            nc.vector.tensor_mul(t0[:], px[0][64:112, :], s_sb[0:48, :])
            for b in range(2):
                nc.tensor.matmul(px[0][0:48, 512 * b:512 * b + 512],
                                 w_blk(3, slice(0, 48), 48),
                                 t0[:, 512 * b:512 * b + 512],
                                 start=False, stop=True, skip_group_check=True)
            nc.scalar.activation(o_sb[0:48, ocol:ocol + 1024], px[0][0:48, :],
                                 mybir.ActivationFunctionType.Copy)
            # quarter C (h1 cols 0:cs): fully on DVE
            cs = CSPL_TAIL if pair == 7 else CSPL
            t1 = tpool.tile([48, cs], f16, name="t1")
            nc.vector.tensor_mul(t1[:], px[1][64:112, 0:cs],
                                 s_sb[64:112, 0:cs])
            nc.vector.tensor_add(o_sb[64:112, ocol:ocol + cs],
                                 px[1][0:48, 0:cs], t1[:])

        # per-pair output tiles: avoids cross-pair writer chaining on a
        # shared tile (the framework serializes same-tile accessors)
        o_tiles = [(opool.tile([112, 1024], f16, name=f"o{k}"), [k])
                   for k in range(8)]
        pair_otile = {}
        for o_sb, pairs in o_tiles:
            for i, p in enumerate(pairs):
                pair_otile[p] = (o_sb, i * 1024, p == pairs[-1])

        def store(o_sb, pairs):
            b0, nb = 2 * pairs[0], 2 * len(pairs)
            for r0, y0 in ((0, 0), (64, 48)):
                nc.sync.dma_start(
                    yp[b0:b0 + nb, y0:y0 + 48].rearrange("b p f -> p b f"),
                    o_sb[r0:r0 + 48, :].rearrange("p (b f) -> p b f", b=nb))

        s_store = {}
        for p in range(9):
            px = None
            if p >= 1:
                # allocation order [px0, gate, px1] makes the pool-slot reuse
                # edges land on early-freed tiles: px1(p) <- gate(p) (freed by
                # the sigmoid), gate(p+1) <- px0(p) (copy_h0, but gates have a
                # pair of lead slack), px0(p) <- px1(p-1) (freed mid-combine)
                px = [xps.tile([112, 1024], f32, name=f"px{p-1}_0", tag="px"),
                      None]
                xpath_half(p - 1, px, 0)
            if p < 8:
                s_store[p] = gates_stage(p)
            if p >= 1:
                px[1] = xps.tile([112, 1024], f32, name=f"px{p-1}_1", tag="px")
                xpath_half(p - 1, px, 1)
                o_sb, ocol, last = pair_otile[p - 1]
                s_prev = s_store.pop(p - 1)
                combine_d(p - 1, px, s_prev, o_sb, ocol)
                combine_rest(p - 1, px, s_prev, o_sb, ocol)
                if last:
                    store(*[ot for ot in o_tiles if p - 1 in ot[1]][0])

    nc.compile()
    nc.m = get_hw_module(nc.m)
    return nc


_PROGRAMS = {}


def _get_program(fold):
    if fold not in _PROGRAMS:
        _PROGRAMS[fold] = build_program(fold)
    return _PROGRAMS[fold]


def pack_inputs(inp, ctrl, bparam, W, hi, lo, fold):
    """Host-side shard + layout packing. Returns in_maps (list of 8 dicts)."""
    import ml_dtypes
    X = np.asarray(inp, np.float32).reshape(B, 2, 4, 512, NDW * LL)
    # [B, h, g', f, k] -> [B, h, g'*nk + k, f]
    Xhi = np.ascontiguousarray(
        X[..., hi].transpose(0, 1, 2, 4, 3)).reshape(B, 2, 4 * N_HI, 512)
    Xlo = np.ascontiguousarray(
        X[..., lo].transpose(0, 1, 2, 4, 3)).reshape(B, 2, 4 * N_LO, 512)
    ctf = np.asarray(ctrl, np.float64)
    if fold:
        binv = np.asarray(bparam, np.float64) @ np.linalg.inv(
            np.asarray(W, np.float64))
        ctf = ctf + binv[None, :, :]
    CT = np.ascontiguousarray(
        ctf.astype(np.float32).reshape(B, NG, 512, 12).transpose(0, 1, 3, 2)
    ).reshape(B, 96, 512)
    Xhi = Xhi.astype(np.float16)
    Xlo = Xlo.astype(ml_dtypes.float8_e3m4)
    CT = CT.astype(np.float16)
    in_maps = []
    for c in range(N_CORES):
        sl = slice(c * BPC, (c + 1) * BPC)
        in_maps.append({"xhi": Xhi[sl], "xlo": Xlo[sl], "ct": CT[sl]})
    return in_maps


def unpack_output(results):
    yp = np.concatenate([r["yp"].astype(np.float32) for r in results], axis=0)
    return np.ascontiguousarray(
        yp.reshape(B, NG, 12, 512).transpose(0, 1, 3, 2)
    ).reshape(B, NNODES, NPRED)


def kernel(inp, ctrl, offset_t, offset_n, conv_t_w, conv_t_b, conv_n_w,
           conv_n_b, W, bparam):
    from concourse.bass_utils import run_bass_kernel_spmd

    hi, hi_mask, lo, lo_mask, fold, A_t, D = _plan(
        offset_t, offset_n, conv_t_w, conv_n_w, W)
    nc = _get_program(fold)
    wts_np = _build_weights(hi, hi_mask, lo, lo_mask, fold, A_t, D, W)
    in_maps = pack_inputs(inp, ctrl, bparam, W, hi, lo, fold)
    for m in in_maps:
        m["wts"] = wts_np.astype(np.float16)
        if not fold:
            bias_t = np.ascontiguousarray(
                np.asarray(bparam, np.float32).reshape(NG, 512, 12)
                .transpose(0, 2, 1)).reshape(96, 512)
            m["bias"] = bias_t.astype(np.float16)
    res = run_bass_kernel_spmd(nc, in_maps, core_ids=list(range(N_CORES)))
    out = unpack_output(res.results)
    # Conv biases are zeros in this module's init, so the device kernel omits
    # them. If ever nonzero, apply the exact correction on the host.
    ctb = float(np.asarray(conv_t_b).reshape(-1)[0])
    cnb = np.asarray(conv_n_b, np.float32)
    if ctb != 0.0 or np.any(cnb != 0.0):
        G = np.asarray(ctrl, np.float32).reshape(B * NNODES, NPRED) @ np.asarray(
            W, np.float32)
        G += np.tile(np.asarray(bparam, np.float32), (B, 1))
        S = 1.0 / (1.0 + np.exp(-G))
        out = out + (ctb + (cnb[None, :] - ctb) * S).reshape(B, NNODES, NPRED)
    return out.astype(np.float32)


# revision 47
# speedup vs baseline: 1.0815x; 1.0533x over previous
"""Trainium2 Bass kernel for nn_DeformableConvStandard.

The deformable interpolation + both convs are linear in `inp` once the
(tiny) offsets are known, so the whole module collapses to

    out = Pt + Pd * sigmoid(ctrl' @ W),   Pt = X @ A_t,  Pd = X @ D

with A_t, D: [48, 12] host-built from offsets/conv weights, and the gate
bias pre-folded into ctrl' = ctrl + bparam @ W^-1 on the host. Rows of
[A_t|D] that are identically zero (deform positions never sampled) are
pruned from the streamed X features.

Per-core layout (16 batches = 8 pairs of 2): each pair's x-path runs in
two 2-bank PSUM tiles [112, 1024] (cols = batch), one per group-half h,
with Pt at partitions 0-47 and Pd at 64-111 (48-part blocks may only
start at 0/64). The streamed features split by weight sensitivity into
12 fp16 ("hi") + 32 fp8-e3m4 ("lo") contraction chunks, which cuts the
dominant DMA stream by 30% at ~0.9e-2 rel err. The combine
out = Pt + Pd*S is spread over all four compute engines per pair:
  - one merged DVE mult [48,1024] for half h0, two PE identity-add
    matmuls, one merged ACT copy [48,1024] to the fp16 out tile;
  - quarter C (batch0, h1) fully on DVE ([48,512] mult+add);
  - quarter D (batch1, h1): ACT copies the PSUM tile to SBUF, then the
    (otherwise idle) GPSIMD engine does mult+add SBUF-only.
One merged sigmoid [112,1024] per pair (gate PSUM tile spans 2 banks).
All input DMAs are hoisted up-front on the sync queue; output stores
follow on the same queue (2-pair output tiles, last 2 pairs split for a
shorter tail).
"""
import numpy as np
from contextlib import ExitStack

N_CORES = 8
B, NNODES = 128, 4096
NDW, LL, NPRED, NDRIFT = 3, 16, 12, 2
BPC = B // N_CORES          # batches per core: 16
NG = NNODES // 512          # node groups per batch: 8
N_HI = 12                   # fp16-streamed features (4*N_HI <= 128)
N_LO = 32                   # fp8e3m4-streamed features (4*N_LO <= 128)
MAX_W_COND = 1e4            # fold bias into ctrl only if W is this well-posed
N_WARM_MM = 8              # dummy PE warm-up matmuls (p-state ramp + fill)
WARM_FREE = 400             # free size of each warm-up matmul


def _build_A(offset, conv_w, mode):
    """A [48, 12] with pred = X @ A for X [rows, 48], feature = d*16+l."""
    off = np.asarray(offset, np.float32)
    pos = np.tanh(off) * np.float32(NDRIFT) + (
        np.arange(NPRED, dtype=np.float32) + np.float32(NDRIFT)
    )[None, :]
    key = np.floor(pos)
    frac = (pos - key).astype(np.float64)
    idx = key.astype(np.int32)
    M = np.zeros((NDW, LL, NPRED), np.float64)
    for d in range(NDW):
        for j in range(NPRED):
            M[d, idx[d, j], j] += 1.0 - frac[d, j]
            M[d, idx[d, j] + 1, j] += frac[d, j]
    A = np.zeros((NDW, LL, NPRED), np.float64)
    w = np.asarray(conv_w, np.float64)
    if mode == "t":
        for p in range(NPRED):
            for k in range(3):
                j = p + k - 1
                if 0 <= j < NPRED:
                    A[:, :, p] += w[0, :, k][:, None] * M[:, :, j]
    else:
        for o in range(NPRED):
            for d in range(NDW):
                for c in range(NPRED):
                    A[d, :, o] += w[o, c, d] * M[d, :, c]
    return A.reshape(NDW * LL, NPRED)


def _plan(offset_t, offset_n, conv_t_w, conv_n_w, W):
    """Split used features into hi (fp16) / lo (fp8) index lists."""
    A_t = _build_A(offset_t, conv_t_w, "t")
    D = _build_A(offset_n, conv_n_w, "n") - A_t
    c2 = (A_t ** 2 + D ** 2).sum(axis=1)
    used = np.where(c2 > 0)[0]
    assert len(used) <= N_HI + N_LO, f"{len(used)} used features > {N_HI + N_LO}"
    n_hi = max(N_HI, len(used) - N_LO)
    by_weight = used[np.argsort(-c2[used])]
    hi = np.sort(by_weight[:n_hi])
    lo = np.sort(by_weight[n_hi:])
    lo_mask = np.ones(N_LO)
    lo_mask[len(lo):] = 0.0
    hi_mask = np.ones(N_HI)
    hi_mask[len(hi):] = 0.0
    hi = np.concatenate([hi, np.zeros(N_HI - len(hi), np.int64)])
    lo = np.concatenate([lo, np.zeros(N_LO - len(lo), np.int64)])
    Wf = np.asarray(W, np.float64)
    fold = np.linalg.cond(Wf) < MAX_W_COND
    return hi, hi_mask, lo, lo_mask, fold, A_t, D


def _build_weights(hi, hi_mask, lo, lo_mask, fold, A_t, D, W):
    """Stationary lhsT blocks packed as one [128, NW*112] tensor.

    block 0: gate blockdiag-W [96, 112]; block 1: hi lhsT [48, 112];
    block 2: lo lhsT [128, 112]; block 3: I48; block 4 (no-fold only):
    bias-permutation identity [96, 112]. x-path cols: Pt at 12g+j,
    Pd at 64+12g+j for group-slot g in 0..3.
    """
    NW = 5 if not fold else 4
    wts = np.zeros((128, NW * 112), np.float64)
    Wf = np.asarray(W, np.float64)
    for g in range(8):
        col = 12 * (g % 4) + 64 * (g // 4)
        wts[12 * g:12 * g + 12, col:col + 12] = Wf
        if not fold:
            for q in range(12):
                wts[12 * g + q, 4 * 112 + col + q] = 1.0
    A_hi = A_t[hi] * hi_mask[:, None]
    D_hi = D[hi] * hi_mask[:, None]
    A_lo = A_t[lo] * lo_mask[:, None]
    D_lo = D[lo] * lo_mask[:, None]
    for g in range(4):
        c = 112 + 12 * g
        wts[N_HI * g:N_HI * (g + 1), c:c + 12] = A_hi
        wts[N_HI * g:N_HI * (g + 1), c + 64:c + 76] = D_hi
        c = 224 + 12 * g
        wts[N_LO * g:N_LO * (g + 1), c:c + 12] = A_lo
        wts[N_LO * g:N_LO * (g + 1), c + 64:c + 76] = D_lo
    wts[0:48, 336:384] = np.eye(48)
    return wts.astype(np.float32)


def build_program(fold=True):
    import concourse.bass as bass
    import concourse.tile as tile
    from concourse import bacc, mybir
    from concourse.bass_interp import get_hw_module

    f32 = mybir.dt.float32
    f16 = mybir.dt.float16
    e3 = mybir.dt.float8e3
    NW = 4 if fold else 5

    nc = bacc.Bacc("TRN2", target_bir_lowering=False, debug=False,
                   num_devices=N_CORES)
    xhi = nc.dram_tensor("xhi", [BPC, 2, 4 * N_HI, 512], f16,
                         kind="ExternalInput").ap()
    xlo = nc.dram_tensor("xlo", [BPC, 2, 4 * N_LO, 512], e3,
                         kind="ExternalInput").ap()
    ct = nc.dram_tensor("ct", [BPC, 96, 512], f16, kind="ExternalInput").ap()
    wts = nc.dram_tensor("wts", [128, NW * 112], f16, kind="ExternalInput").ap()
    if not fold:
        bias = nc.dram_tensor("bias", [96, 512], f16, kind="ExternalInput").ap()
    yp = nc.dram_tensor("yp", [BPC, 96, 512], f16, kind="ExternalOutput").ap()

    with tile.TileContext(nc) as tc, ExitStack() as ctx:
        consts = ctx.enter_context(tc.tile_pool(name="consts", bufs=1))
        xpool = ctx.enter_context(tc.tile_pool(name="xp", bufs=1))
        cpool = ctx.enter_context(tc.tile_pool(name="ct", bufs=1))
        spool = ctx.enter_context(tc.tile_pool(name="sig", bufs=4))
        tpool = ctx.enter_context(tc.tile_pool(name="tmp", bufs=8))
        opool = ctx.enter_context(tc.tile_pool(name="ost", bufs=1))
        xps = ctx.enter_context(
            tc.tile_pool(name="xps", bufs=4, space=bass.MemorySpace.PSUM))
        gps = xps

        # ---- all input DMAs up-front, in pipeline order ----
        w_sb = consts.tile([128, NW * 112], f16)
        nc.sync.dma_start(w_sb[:], wts[:])
        if not fold:
            bias_sb = consts.tile([96, 512], f16)
            nc.sync.dma_start(bias_sb[:], bias[:])

        ct_tiles = [cpool.tile([96, 2048], f16, name=f"ct{i}") for i in range(4)]
        xhi_tiles = [None] * 8
        xlo_tiles = [None] * 8

        def load_ct(i4, half=None):
            # half granularity for the first tile (faster pipeline fill)
            b0 = i4 * 4 if half is None else i4 * 4 + half * 2
            nb = 4 if half is None else 2
            c0 = 0 if half in (None, 0) else 1024
            # ct goes through the Pool SWDGE path: no HWDGE hold, and the
            # GPSIMD engine is idle during the fill when these issue
            nc.gpsimd.dma_start(
                ct_tiles[i4][:, c0:c0 + nb * 512]
                .rearrange("p (b f) -> p b f", b=nb),
                ct[b0:b0 + nb].rearrange("b p f -> p b f"))

        def load_x(pair0, npair, split_batches=False):
            b0 = pair0 * 2
            thi = xpool.tile([4 * N_HI, npair * 2048], f16, name=f"xh{pair0}")
            tlo = xpool.tile([4 * N_LO, npair * 2048], e3, name=f"xl{pair0}")
            if split_batches:
                for b in range(2 * npair):
                    nc.sync.dma_start(
                        thi[:, b * 1024:(b + 1) * 1024]
                        .rearrange("p (h f) -> p h f", h=2),
                        xhi[b0 + b])
                    nc.sync.dma_start(
                        tlo[:, b * 1024:(b + 1) * 1024]
                        .rearrange("p (h f) -> p h f", h=2),
                        xlo[b0 + b])
            else:
                nc.sync.dma_start(
                    thi[:].rearrange("p (b h f) -> p b h f", b=2 * npair, h=2),
                    xhi[b0:b0 + 2 * npair].rearrange("b h p f -> p b h f"))
                nc.sync.dma_start(
                    tlo[:].rearrange("p (b h f) -> p b h f", b=2 * npair, h=2),
                    xlo[b0:b0 + 2 * npair].rearrange("b h p f -> p b h f"))
            for k in range(npair):
                xhi_tiles[pair0 + k] = (thi, k * 2048)
                xlo_tiles[pair0 + k] = (tlo, k * 2048)

        load_x(0, 1)
        load_ct(0, 0)
        load_x(1, 1)
        load_ct(0, 1)
        for i4 in range(1, 4):
            load_ct(i4)
            load_x(i4 * 2, 1)
            load_x(i4 * 2 + 1, 1)

        def w_blk(k, rows, ncols=112):
            return w_sb[rows, k * 112:k * 112 + ncols]

        # warm-up matmuls: keep PE busy through the DMA fill so the p-state
        # ramp completes before the first real matmul
        wp = xps.tile([112, 1024], f32, name="warm", tag="px")
        for i in range(N_WARM_MM):
            nc.tensor.matmul(wp[0:16, 0:WARM_FREE], w_sb[0:16, 0:16],
                             w_sb[0:16, 0:WARM_FREE], start=True, stop=True)


        # ---- per-pair stages; gates run one pair ahead ----
        def gates_stage(pair):
            i4, half = divmod(pair, 2)
            g_ps = gps.tile([112, 1024], f32, name=f"g{pair}", tag="px")
            for b in range(2):
                cs = 1024 * half + 512 * b
                nc.tensor.matmul(g_ps[:, 512 * b:512 * b + 512],
                                 w_blk(0, slice(0, 96)),
                                 ct_tiles[i4][:, cs:cs + 512],
                                 start=True, stop=fold)
            if not fold:
                for b in range(2):
                    nc.tensor.matmul(g_ps[:, 512 * b:512 * b + 512],
                                     w_blk(4, slice(0, 96)), bias_sb[:],
                                     start=False, stop=True)
            s_sb = spool.tile([112, 1024], f16)
            nc.scalar.activation(s_sb[:], g_ps[:],
                                 mybir.ActivationFunctionType.Sigmoid)
            return s_sb

        def xpath_half(pair, px, h):
            thi, hoff = xhi_tiles[pair]
            tlo, loff = xlo_tiles[pair]
            for b in range(2):
                xc = (2 * b + h) * 512
                out = px[h][:, 512 * b:512 * b + 512]
                nc.tensor.matmul(out, w_blk(1, slice(0, 4 * N_HI)),
                                 thi[:, hoff + xc:hoff + xc + 512],
                                 start=True, stop=False)
                nc.tensor.matmul(out, w_blk(2, slice(0, 4 * N_LO)),
                                 tlo[:, loff + xc:loff + xc + 512],
                                 start=False, stop=True)

        def combine_d(pair, px, s_sb, o_sb, ocol):
            # quarter D (h1 cols cs:1024) first: its ACT copy only needs
            # the x-path, and the GPSIMD chain behind it is the longest
            cs = CSPL_TAIL if pair == 7 else CSPL
            dw = 1024 - cs
            d_sb = tpool.tile([112, dw], f16, name="d")
            nc.scalar.activation(d_sb[:], px[1][:, cs:1024],
                                 mybir.ActivationFunctionType.Copy)
            td = tpool.tile([48, dw], f16, name="td")
            nc.gpsimd.tensor_mul(td[:], d_sb[64:112, :],
                                 s_sb[64:112, cs:1024])
            nc.gpsimd.tensor_add(o_sb[64:112, ocol + cs:ocol + 1024],
                                 d_sb[0:48, :], td[:])

        def combine_rest(pair, px, s_sb, o_sb, ocol):
            # half h0: merged DVE mult, PE identity-adds, merged ACT copy
            t0 = tpool.tile([48, 1024], f16, name="t0")
            nc.vector.tensor_mul(t0[:], px[0][64:112, :], s_sb[0:48, :])
            for b in range(2):
                nc.tensor.matmul(px[0][0:48, 512 * b:512 * b + 512],
                                 w_blk(3, slice(0, 48), 48),
                                 t0[:, 512 * b:512 * b + 512],
                                 start=False, stop=True, skip_group_check=True)
            nc.scalar.activation(o_sb[0:48, ocol:ocol + 1024], px[0][0:48, :],
                                 mybir.ActivationFunctionType.Copy)
            # quarter C (h1 cols 0:cs): fully on DVE
            cs = CSPL_TAIL if pair == 7 else CSPL
            t1 = tpool.tile([48, cs], f16, name="t1")
            nc.vector.tensor_mul(t1[:], px[1][64:112, 0:cs],
                                 s_sb[64:112, 0:cs])
            nc.vector.tensor_add(o_sb[64:112, ocol:ocol + cs],
                                 px[1][0:48, 0:cs], t1[:])

        # per-pair output tiles: avoids cross-pair writer chaining on a
        # shared tile (the framework serializes same-tile accessors)
        o_tiles = [(opool.tile([112, 1024], f16, name=f"o{k}"), [k])
                   for k in range(8)]
        pair_otile = {}
        for o_sb, pairs in o_tiles:
            for i, p in enumerate(pairs):
                pair_otile[p] = (o_sb, i * 1024, p == pairs[-1])

        def store(o_sb, pairs):
            b0, nb = 2 * pairs[0], 2 * len(pairs)
            for r0, y0 in ((0, 0), (64, 48)):
                nc.sync.dma_start(
                    yp[b0:b0 + nb, y0:y0 + 48].rearrange("b p f -> p b f"),
                    o_sb[r0:r0 + 48, :].rearrange("p (b f) -> p b f", b=nb))

        s_store = {}
        for p in range(9):
            px = None
            if p >= 1:
                # allocation order [px0, gate, px1] makes the pool-slot reuse
                # edges land on early-freed tiles: px1(p) <- gate(p) (freed by
                # the sigmoid), gate(p+1) <- px0(p) (copy_h0, but gates have a
                # pair of lead slack), px0(p) <- px1(p-1) (freed mid-combine)
                px = [xps.tile([112, 1024], f32, name=f"px{p-1}_0", tag="px"),
                      None]
                xpath_half(p - 1, px, 0)
            if p < 8:
                s_store[p] = gates_stage(p)
            if p >= 1:
                px[1] = xps.tile([112, 1024], f32, name=f"px{p-1}_1", tag="px")
                xpath_half(p - 1, px, 1)
                o_sb, ocol, last = pair_otile[p - 1]
                s_prev = s_store.pop(p - 1)
                combine_d(p - 1, px, s_prev, o_sb, ocol)
                combine_rest(p - 1, px, s_prev, o_sb, ocol)
                if last:
                    store(*[ot for ot in o_tiles if p - 1 in ot[1]][0])

    nc.compile()
    nc.m = get_hw_module(nc.m)
    return nc


_PROGRAMS = {}


def _get_program(fold):
    if fold not in _PROGRAMS:
        _PROGRAMS[fold] = build_program(fold)
    return _PROGRAMS[fold]


def pack_inputs(inp, ctrl, bparam, W, hi, lo, fold):
    """Host-side shard + layout packing. Returns in_maps (list of 8 dicts)."""
    import ml_dtypes
    X = np.asarray(inp, np.float32).reshape(B, 2, 4, 512, NDW * LL)
    # [B, h, g', f, k] -> [B, h, g'*nk + k, f]
    Xhi = np.ascontiguousarray(
        X[..., hi].transpose(0, 1, 2, 4, 3)).reshape(B, 2, 4 * N_HI, 512)
    Xlo = np.ascontiguousarray(
        X[..., lo].transpose(0, 1, 2, 4, 3)).reshape(B, 2, 4 * N_LO, 512)
    ctf = np.asarray(ctrl, np.float64)
    if fold:
        binv = np.asarray(bparam, np.float64) @ np.linalg.inv(
            np.asarray(W, np.float64))
        ctf = ctf + binv[None, :, :]
    CT = np.ascontiguousarray(
        ctf.astype(np.float32).reshape(B, NG, 512, 12).transpose(0, 1, 3, 2)
    ).reshape(B, 96, 512)
    Xhi = Xhi.astype(np.float16)
    Xlo = Xlo.astype(ml_dtypes.float8_e3m4)
    CT = CT.astype(np.float16)
    in_maps = []
    for c in range(N_CORES):
        sl = slice(c * BPC, (c + 1) * BPC)
        in_maps.append({"xhi": Xhi[sl], "xlo": Xlo[sl], "ct": CT[sl]})
    return in_maps


def unpack_output(results):
    yp = np.concatenate([r["yp"].astype(np.float32) for r in results], axis=0)
    return np.ascontiguousarray(
        yp.reshape(B, NG, 12, 512).transpose(0, 1, 3, 2)
    ).reshape(B, NNODES, NPRED)


def kernel(inp, ctrl, offset_t, offset_n, conv_t_w, conv_t_b, conv_n_w,
           conv_n_b, W, bparam):
    from concourse.bass_utils import run_bass_kernel_spmd

    hi, hi_mask, lo, lo_mask, fold, A_t, D = _plan(
        offset_t, offset_n, conv_t_w, conv_n_w, W)
    nc = _get_program(fold)
    wts_np = _build_weights(hi, hi_mask, lo, lo_mask, fold, A_t, D, W)
    in_maps = pack_inputs(inp, ctrl, bparam, W, hi, lo, fold)
    for m in in_maps:
        m["wts"] = wts_np.astype(np.float16)
        if not fold:
            bias_t = np.ascontiguousarray(
                np.asarray(bparam, np.float32).reshape(NG, 512, 12)
                .transpose(0, 2, 1)).reshape(96, 512)
            m["bias"] = bias_t.astype(np.float16)
    res = run_bass_kernel_spmd(nc, in_maps, core_ids=list(range(N_CORES)))
    out = unpack_output(res.results)
    # Conv biases are zeros in this module's init, so the device kernel omits
    # them. If ever nonzero, apply the exact correction on the host.
    ctb = float(np.asarray(conv_t_b).reshape(-1)[0])
    cnb = np.asarray(conv_n_b, np.float32)
    if ctb != 0.0 or np.any(cnb != 0.0):
        G = np.asarray(ctrl, np.float32).reshape(B * NNODES, NPRED) @ np.asarray(
            W, np.float32)
        G += np.tile(np.asarray(bparam, np.float32), (B, 1))
        S = 1.0 / (1.0 + np.exp(-G))
        out = out + (ctb + (cnb[None, :] - ctb) * S).reshape(B, NNODES, NPRED)
    return out.astype(np.float32)


# revision 55
# speedup vs baseline: 1.0921x; 1.0098x over previous
"""Trainium2 Bass kernel for nn_DeformableConvStandard.

The deformable interpolation + both convs are linear in `inp` once the
(tiny) offsets are known, so the whole module collapses to

    out = Pt + Pd * sigmoid(ctrl' @ W),   Pt = X @ A_t,  Pd = X @ D

with A_t, D: [48, 12] host-built from offsets/conv weights, and the gate
bias pre-folded into ctrl' = ctrl + bparam @ W^-1 on the host. Rows of
[A_t|D] that are identically zero (deform positions never sampled) are
pruned from the streamed X features.

Per-core layout (16 batches = 8 pairs of 2): each pair's x-path runs in
two 2-bank PSUM tiles [112, 1024] (cols = batch), one per group-half h,
with Pt at partitions 0-47 and Pd at 64-111 (48-part blocks may only
start at 0/64). The streamed features split by weight sensitivity into
12 fp16 ("hi") + 32 fp8-e3m4 ("lo") contraction chunks, which cuts the
dominant DMA stream by 30% at ~0.9e-2 rel err. The combine
out = Pt + Pd*S is spread over all four compute engines per pair:
  - one merged DVE mult [48,1024] for half h0, two PE identity-add
    matmuls, one merged ACT copy [48,1024] to the fp16 out tile;
  - quarter C (batch0, h1) fully on DVE ([48,512] mult+add);
  - quarter D (batch1, h1): ACT copies the PSUM tile to SBUF, then the
    (otherwise idle) GPSIMD engine does mult+add SBUF-only.
One merged sigmoid [112,1024] per pair (gate PSUM tile spans 2 banks).
All input DMAs are hoisted up-front on the sync queue; output stores
follow on the same queue (2-pair output tiles, last 2 pairs split for a
shorter tail).
"""
import numpy as np
from contextlib import ExitStack

N_CORES = 8
B, NNODES = 128, 4096
NDW, LL, NPRED, NDRIFT = 3, 16, 12, 2
BPC = B // N_CORES          # batches per core: 16
NG = NNODES // 512          # node groups per batch: 8
N_HI = 12                   # fp16-streamed features (4*N_HI <= 128)
N_LO = 32                   # fp8e3m4-streamed features (4*N_LO <= 128)
MAX_W_COND = 1e4            # fold bias into ctrl only if W is this well-posed
N_WARM_MM = 8              # dummy PE warm-up matmuls (p-state ramp + fill)
WARM_FREE = 400             # free size of each warm-up matmul


def _build_A(offset, conv_w, mode):
    """A [48, 12] with pred = X @ A for X [rows, 48], feature = d*16+l."""
    off = np.asarray(offset, np.float32)
    pos = np.tanh(off) * np.float32(NDRIFT) + (
        np.arange(NPRED, dtype=np.float32) + np.float32(NDRIFT)
    )[None, :]
    key = np.floor(pos)
    frac = (pos - key).astype(np.float64)
    idx = key.astype(np.int32)
    M = np.zeros((NDW, LL, NPRED), np.float64)
    for d in range(NDW):
        for j in range(NPRED):
            M[d, idx[d, j], j] += 1.0 - frac[d, j]
            M[d, idx[d, j] + 1, j] += frac[d, j]
    A = np.zeros((NDW, LL, NPRED), np.float64)
    w = np.asarray(conv_w, np.float64)
    if mode == "t":
        for p in range(NPRED):
            for k in range(3):
                j = p + k - 1
                if 0 <= j < NPRED:
                    A[:, :, p] += w[0, :, k][:, None] * M[:, :, j]
    else:
        for o in range(NPRED):
            for d in range(NDW):
                for c in range(NPRED):
                    A[d, :, o] += w[o, c, d] * M[d, :, c]
    return A.reshape(NDW * LL, NPRED)


def _plan(offset_t, offset_n, conv_t_w, conv_n_w, W):
    """Split used features into hi (fp16) / lo (fp8) index lists."""
    A_t = _build_A(offset_t, conv_t_w, "t")
    D = _build_A(offset_n, conv_n_w, "n") - A_t
    c2 = (A_t ** 2 + D ** 2).sum(axis=1)
    used = np.where(c2 > 0)[0]
    assert len(used) <= N_HI + N_LO, f"{len(used)} used features > {N_HI + N_LO}"
    n_hi = max(N_HI, len(used) - N_LO)
    by_weight = used[np.argsort(-c2[used])]
    hi = np.sort(by_weight[:n_hi])
    lo = np.sort(by_weight[n_hi:])
    lo_mask = np.ones(N_LO)
    lo_mask[len(lo):] = 0.0
    hi_mask = np.ones(N_HI)
    hi_mask[len(hi):] = 0.0
    hi = np.concatenate([hi, np.zeros(N_HI - len(hi), np.int64)])
    lo = np.concatenate([lo, np.zeros(N_LO - len(lo), np.int64)])
    Wf = np.asarray(W, np.float64)
    fold = np.linalg.cond(Wf) < MAX_W_COND
    return hi, hi_mask, lo, lo_mask, fold, A_t, D


def _build_weights(hi, hi_mask, lo, lo_mask, fold, A_t, D, W):
    """Stationary lhsT blocks packed as one [128, NW*112] tensor.

    block 0: gate blockdiag-W [96, 112]; block 1: hi lhsT [48, 112];
    block 2: lo lhsT [128, 112]; block 3: I48; block 4 (no-fold only):
    bias-permutation identity [96, 112]. x-path cols: Pt at 12g+j,
    Pd at 64+12g+j for group-slot g in 0..3.
    """
    NW = 5 if not fold else 4
    wts = np.zeros((128, NW * 112), np.float64)
    Wf = np.asarray(W, np.float64)
    for g in range(8):
        col = 12 * (g % 4) + 64 * (g // 4)
        wts[12 * g:12 * g + 12, col:col + 12] = Wf
        if not fold:
            for q in range(12):
                wts[12 * g + q, 4 * 112 + col + q] = 1.0
    A_hi = A_t[hi] * hi_mask[:, None]
    D_hi = D[hi] * hi_mask[:, None]
    A_lo = A_t[lo] * lo_mask[:, None]
    D_lo = D[lo] * lo_mask[:, None]
    for g in range(4):
        c = 112 + 12 * g
        wts[N_HI * g:N_HI * (g + 1), c:c + 12] = A_hi
        wts[N_HI * g:N_HI * (g + 1), c + 64:c + 76] = D_hi
        c = 224 + 12 * g
        wts[N_LO * g:N_LO * (g + 1), c:c + 12] = A_lo
        wts[N_LO * g:N_LO * (g + 1), c + 64:c + 76] = D_lo
    wts[0:48, 336:384] = np.eye(48)
    return wts.astype(np.float32)


def build_program(fold=True):
    import concourse.bass as bass
    import concourse.tile as tile
    from concourse import bacc, mybir
    from concourse.bass_interp import get_hw_module

    f32 = mybir.dt.float32
    f16 = mybir.dt.float16
    e3 = mybir.dt.float8e3
    NW = 4 if fold else 5

    nc = bacc.Bacc("TRN2", target_bir_lowering=False, debug=False,
                   num_devices=N_CORES)
    xhi = nc.dram_tensor("xhi", [BPC, 2, 4 * N_HI, 512], f16,
                         kind="ExternalInput").ap()
    xlo = nc.dram_tensor("xlo", [BPC, 2, 4 * N_LO, 512], e3,
                         kind="ExternalInput").ap()
    ct = nc.dram_tensor("ct", [BPC, 96, 512], f16, kind="ExternalInput").ap()
    wts = nc.dram_tensor("wts", [128, NW * 112], f16, kind="ExternalInput").ap()
    if not fold:
        bias = nc.dram_tensor("bias", [96, 512], f16, kind="ExternalInput").ap()
    yp = nc.dram_tensor("yp", [BPC, 96, 512], f16, kind="ExternalOutput").ap()

    with tile.TileContext(nc) as tc, ExitStack() as ctx:
        consts = ctx.enter_context(tc.tile_pool(name="consts", bufs=1))
        xpool = ctx.enter_context(tc.tile_pool(name="xp", bufs=1))
        cpool = ctx.enter_context(tc.tile_pool(name="ct", bufs=1))
        spool = ctx.enter_context(tc.tile_pool(name="sig", bufs=4))
        tpool = ctx.enter_context(tc.tile_pool(name="tmp", bufs=8))
        opool = ctx.enter_context(tc.tile_pool(name="ost", bufs=1))
        xps = ctx.enter_context(
            tc.tile_pool(name="xps", bufs=4, space=bass.MemorySpace.PSUM))
        gps = xps

        # ---- all input DMAs up-front, in pipeline order ----
        w_sb = consts.tile([128, NW * 112], f16)
        nc.sync.dma_start(w_sb[:], wts[:])
        if not fold:
            bias_sb = consts.tile([96, 512], f16)
            nc.sync.dma_start(bias_sb[:], bias[:])

        ct_tiles = [cpool.tile([96, 2048], f16, name=f"ct{i}") for i in range(4)]
        xhi_tiles = [None] * 8
        xlo_tiles = [None] * 8

        def load_ct(i4, half=None):
            # half granularity for the first tile (faster pipeline fill)
            b0 = i4 * 4 if half is None else i4 * 4 + half * 2
            nb = 4 if half is None else 2
            c0 = 0 if half in (None, 0) else 1024
            # ct goes through the Pool SWDGE path: no HWDGE hold, and the
            # GPSIMD engine is idle during the fill when these issue
            nc.gpsimd.dma_start(
                ct_tiles[i4][:, c0:c0 + nb * 512]
                .rearrange("p (b f) -> p b f", b=nb),
                ct[b0:b0 + nb].rearrange("b p f -> p b f"))

        def load_x(pair0, npair, split_batches=False):
            b0 = pair0 * 2
            thi = xpool.tile([4 * N_HI, npair * 2048], f16, name=f"xh{pair0}")
            tlo = xpool.tile([4 * N_LO, npair * 2048], e3, name=f"xl{pair0}")
            if split_batches:
                for b in range(2 * npair):
                    nc.sync.dma_start(
                        thi[:, b * 1024:(b + 1) * 1024]
                        .rearrange("p (h f) -> p h f", h=2),
                        xhi[b0 + b])
                    nc.sync.dma_start(
                        tlo[:, b * 1024:(b + 1) * 1024]
                        .rearrange("p (h f) -> p h f", h=2),
                        xlo[b0 + b])
            else:
                nc.sync.dma_start(
                    thi[:].rearrange("p (b h f) -> p b h f", b=2 * npair, h=2),
                    xhi[b0:b0 + 2 * npair].rearrange("b h p f -> p b h f"))
                nc.sync.dma_start(
                    tlo[:].rearrange("p (b h f) -> p b h f", b=2 * npair, h=2),
                    xlo[b0:b0 + 2 * npair].rearrange("b h p f -> p b h f"))
            for k in range(npair):
                xhi_tiles[pair0 + k] = (thi, k * 2048)
                xlo_tiles[pair0 + k] = (tlo, k * 2048)

        load_x(0, 1)
        load_ct(0, 0)
        load_x(1, 1)
        load_ct(0, 1)
        for i4 in range(1, 4):
            load_ct(i4)
            load_x(i4 * 2, 1)
            load_x(i4 * 2 + 1, 1)

        def w_blk(k, rows, ncols=112):
            return w_sb[rows, k * 112:k * 112 + ncols]

        # warm-up matmuls: keep PE busy through the DMA fill so the p-state
        # ramp completes before the first real matmul
        wp = xps.tile([112, 1024], f32, name="warm", tag="px")
        for i in range(N_WARM_MM):
            nc.tensor.matmul(wp[0:16, 0:WARM_FREE], w_sb[0:16, 0:16],
                             w_sb[0:16, 0:WARM_FREE], start=True, stop=True)


        # ---- per-pair stages; gates run one pair ahead ----
        def gates_stage(pair):
            i4, half = divmod(pair, 2)
            g_ps = gps.tile([112, 1024], f32, name=f"g{pair}", tag="px")
            for b in range(2):
                cs = 1024 * half + 512 * b
                nc.tensor.matmul(g_ps[:, 512 * b:512 * b + 512],
                                 w_blk(0, slice(0, 96)),
                                 ct_tiles[i4][:, cs:cs + 512],
                                 start=True, stop=fold)
            if not fold:
                for b in range(2):
                    nc.tensor.matmul(g_ps[:, 512 * b:512 * b + 512],
                                     w_blk(4, slice(0, 96)), bias_sb[:],
                                     start=False, stop=True)
            return g_ps

        def sig_stage(g_ps, pair):
            s_sb = spool.tile([112, 1024], f16, name=f"s{pair}")
            nc.scalar.activation(s_sb[:], g_ps[:],
                                 mybir.ActivationFunctionType.Sigmoid)
            return s_sb

        def xpath_half(pair, px, h):
            thi, hoff = xhi_tiles[pair]
            tlo, loff = xlo_tiles[pair]
            for b in range(2):
                xc = (2 * b + h) * 512
                out = px[h][:, 512 * b:512 * b + 512]
                nc.tensor.matmul(out, w_blk(1, slice(0, 4 * N_HI)),
                                 thi[:, hoff + xc:hoff + xc + 512],
                                 start=True, stop=False)
                nc.tensor.matmul(out, w_blk(2, slice(0, 4 * N_LO)),
                                 tlo[:, loff + xc:loff + xc + 512],
                                 start=False, stop=True)

        def combine_d(pair, px, s_sb, o_sb, ocol):
            # quarter D (h1 cols cs:1024) first: its ACT copy only needs
            # the x-path, and the GPSIMD chain behind it is the longest
            cs = CSPL_TAIL if pair == 7 else CSPL
            dw = 1024 - cs
            d_sb = tpool.tile([112, dw], f16, name="d")
            nc.scalar.activation(d_sb[:], px[1][:, cs:1024],
                                 mybir.ActivationFunctionType.Copy)
            td = tpool.tile([48, dw], f16, name="td")
            nc.gpsimd.tensor_mul(td[:], d_sb[64:112, :],
                                 s_sb[64:112, cs:1024])
            nc.gpsimd.tensor_add(o_sb[64:112, ocol + cs:ocol + 1024],
                                 d_sb[0:48, :], td[:])

        def combine_rest(pair, px, s_sb, o_sb, ocol):
            # half h0: merged DVE mult, PE identity-adds, merged ACT copy
            t0 = tpool.tile([48, 1024], f16, name="t0")
            nc.vector.tensor_mul(t0[:], px[0][64:112, :], s_sb[0:48, :])
            for b in range(2):
                nc.tensor.matmul(px[0][0:48, 512 * b:512 * b + 512],
                                 w_blk(3, slice(0, 48), 48),
                                 t0[:, 512 * b:512 * b + 512],
                                 start=False, stop=True, skip_group_check=True)
            nc.scalar.activation(o_sb[0:48, ocol:ocol + 1024], px[0][0:48, :],
                                 mybir.ActivationFunctionType.Copy)
            # quarter C (h1 cols 0:cs): fully on DVE
            cs = CSPL_TAIL if pair == 7 else CSPL
            t1 = tpool.tile([48, cs], f16, name="t1")
            nc.vector.tensor_mul(t1[:], px[1][64:112, 0:cs],
                                 s_sb[64:112, 0:cs])
            nc.vector.tensor_add(o_sb[64:112, ocol:ocol + cs],
                                 px[1][0:48, 0:cs], t1[:])

        # per-pair output tiles: avoids cross-pair writer chaining on a
        # shared tile (the framework serializes same-tile accessors)
        o_tiles = [(opool.tile([112, 1024], f16, name=f"o{k}"), [k])
                   for k in range(8)]
        pair_otile = {}
        for o_sb, pairs in o_tiles:
            for i, p in enumerate(pairs):
                pair_otile[p] = (o_sb, i * 1024, p == pairs[-1])

        def store(o_sb, pairs):
            b0, nb = 2 * pairs[0], 2 * len(pairs)
            for r0, y0 in ((0, 0), (64, 48)):
                nc.sync.dma_start(
                    yp[b0:b0 + nb, y0:y0 + 48].rearrange("b p f -> p b f"),
                    o_sb[r0:r0 + 48, :].rearrange("p (b f) -> p b f", b=nb))

        s_store = {}
        g_store = {}
        for p in range(9):
            px = None
            if p >= 1:
                # allocation order [px0, gate, px1] makes the pool-slot reuse
                # edges land on early-freed tiles: px1(p) <- gate(p) (freed by
                # the sigmoid), gate(p+1) <- px0(p) (copy_h0, but gates have a
                # pair of lead slack), px0(p) <- px1(p-1) (freed mid-combine)
                px = [xps.tile([112, 1024], f32, name=f"px{p-1}_0", tag="px"),
                      None]
                xpath_half(p - 1, px, 0)
            if p < 8:
                g_store[p] = gates_stage(p)
            if p >= 1:
                px[1] = xps.tile([112, 1024], f32, name=f"px{p-1}_1", tag="px")
                xpath_half(p - 1, px, 1)
                o_sb, ocol, last = pair_otile[p - 1]
                if p == 1:
                    s_store[0] = sig_stage(g_store.pop(0), 0)
                s_prev = s_store.pop(p - 1)
                combine_d(p - 1, px, s_prev, o_sb, ocol)
                # sigmoid for pair p is emitted after the D-copy so the ACT
                # queue never head-blocks the GPSIMD feed
                if p < 8:
                    s_store[p] = sig_stage(g_store.pop(p), p)
                combine_rest(p - 1, px, s_prev, o_sb, ocol)
                if last:
                    store(*[ot for ot in o_tiles if p - 1 in ot[1]][0])

    nc.compile()
    nc.m = get_hw_module(nc.m)
    return nc


_PROGRAMS = {}


def _get_program(fold):
    if fold not in _PROGRAMS:
        _PROGRAMS[fold] = build_program(fold)
    return _PROGRAMS[fold]


def pack_inputs(inp, ctrl, bparam, W, hi, lo, fold):
    """Host-side shard + layout packing. Returns in_maps (list of 8 dicts)."""
    import ml_dtypes
    X = np.asarray(inp, np.float32).reshape(B, 2, 4, 512, NDW * LL)
    # [B, h, g', f, k] -> [B, h, g'*nk + k, f]
    Xhi = np.ascontiguousarray(
        X[..., hi].transpose(0, 1, 2, 4, 3)).reshape(B, 2, 4 * N_HI, 512)
    Xlo = np.ascontiguousarray(
        X[..., lo].transpose(0, 1, 2, 4, 3)).reshape(B, 2, 4 * N_LO, 512)
    ctf = np.asarray(ctrl, np.float64)
    if fold:
        binv = np.asarray(bparam, np.float64) @ np.linalg.inv(
            np.asarray(W, np.float64))
        ctf = ctf + binv[None, :, :]
    CT = np.ascontiguousarray(
        ctf.astype(np.float32).reshape(B, NG, 512, 12).transpose(0, 1, 3, 2)
    ).reshape(B, 96, 512)
    Xhi = Xhi.astype(np.float16)
    Xlo = Xlo.astype(ml_dtypes.float8_e3m4)
    CT = CT.astype(np.float16)
    in_maps = []
    for c in range(N_CORES):
        sl = slice(c * BPC, (c + 1) * BPC)
        in_maps.append({"xhi": Xhi[sl], "xlo": Xlo[sl], "ct": CT[sl]})
    return in_maps


def unpack_output(results):
    yp = np.concatenate([r["yp"].astype(np.float32) for r in results], axis=0)
    return np.ascontiguousarray(
        yp.reshape(B, NG, 12, 512).transpose(0, 1, 3, 2)
    ).reshape(B, NNODES, NPRED)


def kernel(inp, ctrl, offset_t, offset_n, conv_t_w, conv_t_b, conv_n_w,
           conv_n_b, W, bparam):
    from concourse.bass_utils import run_bass_kernel_spmd

    hi, hi_mask, lo, lo_mask, fold, A_t, D = _plan(
        offset_t, offset_n, conv_t_w, conv_n_w, W)
    nc = _get_program(fold)
    wts_np = _build_weights(hi, hi_mask, lo, lo_mask, fold, A_t, D, W)
    in_maps = pack_inputs(inp, ctrl, bparam, W, hi, lo, fold)
    for m in in_maps:
        m["wts"] = wts_np.astype(np.float16)
        if not fold:
            bias_t = np.ascontiguousarray(
                np.asarray(bparam, np.float32).reshape(NG, 512, 12)
                .transpose(0, 2, 1)).reshape(96, 512)
            m["bias"] = bias_t.astype(np.float16)
    res = run_bass_kernel_spmd(nc, in_maps, core_ids=list(range(N_CORES)))
    out = unpack_output(res.results)
    # Conv biases are zeros in this module's init, so the device kernel omits
    # them. If ever nonzero, apply the exact correction on the host.
    ctb = float(np.asarray(conv_t_b).reshape(-1)[0])
    cnb = np.asarray(conv_n_b, np.float32)
    if ctb != 0.0 or np.any(cnb != 0.0):
        G = np.asarray(ctrl, np.float32).reshape(B * NNODES, NPRED) @ np.asarray(
            W, np.float32)
        G += np.tile(np.asarray(bparam, np.float32), (B, 1))
        S = 1.0 / (1.0 + np.exp(-G))
        out = out + (ctb + (cnb[None, :] - ctb) * S).reshape(B, NNODES, NPRED)
    return out.astype(np.float32)


# revision 57
# speedup vs baseline: 1.1045x; 1.0113x over previous
"""Trainium2 Bass kernel for nn_DeformableConvStandard.

The deformable interpolation + both convs are linear in `inp` once the
(tiny) offsets are known, so the whole module collapses to

    out = Pt + Pd * sigmoid(ctrl' @ W),   Pt = X @ A_t,  Pd = X @ D

with A_t, D: [48, 12] host-built from offsets/conv weights, and the gate
bias pre-folded into ctrl' = ctrl + bparam @ W^-1 on the host. Rows of
[A_t|D] that are identically zero (deform positions never sampled) are
pruned from the streamed X features.

Per-core layout (16 batches = 8 pairs of 2): each pair's x-path runs in
two 2-bank PSUM tiles [112, 1024] (cols = batch), one per group-half h,
with Pt at partitions 0-47 and Pd at 64-111 (48-part blocks may only
start at 0/64). The streamed features split by weight sensitivity into
12 fp16 ("hi") + 32 fp8-e3m4 ("lo") contraction chunks, which cuts the
dominant DMA stream by 30% at ~0.9e-2 rel err. The combine
out = Pt + Pd*S is spread over all four compute engines per pair:
  - one merged DVE mult [48,1024] for half h0, two PE identity-add
    matmuls, one merged ACT copy [48,1024] to the fp16 out tile;
  - quarter C (batch0, h1) fully on DVE ([48,512] mult+add);
  - quarter D (batch1, h1): ACT copies the PSUM tile to SBUF, then the
    (otherwise idle) GPSIMD engine does mult+add SBUF-only.
One merged sigmoid [112,1024] per pair (gate PSUM tile spans 2 banks).
All input DMAs are hoisted up-front on the sync queue; output stores
follow on the same queue (2-pair output tiles, last 2 pairs split for a
shorter tail).
"""
import numpy as np
from contextlib import ExitStack

N_CORES = 8
B, NNODES = 128, 4096
NDW, LL, NPRED, NDRIFT = 3, 16, 12, 2
BPC = B // N_CORES          # batches per core: 16
NG = NNODES // 512          # node groups per batch: 8
N_HI = 12                   # fp16-streamed features (4*N_HI <= 128)
N_LO = 32                   # fp8e3m4-streamed features (4*N_LO <= 128)
MAX_W_COND = 1e4            # fold bias into ctrl only if W is this well-posed
N_WARM_MM = 8              # dummy PE warm-up matmuls (p-state ramp + fill)
WARM_FREE = 400             # free size of each warm-up matmul


def _build_A(offset, conv_w, mode):
    """A [48, 12] with pred = X @ A for X [rows, 48], feature = d*16+l."""
    off = np.asarray(offset, np.float32)
    pos = np.tanh(off) * np.float32(NDRIFT) + (
        np.arange(NPRED, dtype=np.float32) + np.float32(NDRIFT)
    )[None, :]
    key = np.floor(pos)
    frac = (pos - key).astype(np.float64)
    idx = key.astype(np.int32)
    M = np.zeros((NDW, LL, NPRED), np.float64)
    for d in range(NDW):
        for j in range(NPRED):
            M[d, idx[d, j], j] += 1.0 - frac[d, j]
            M[d, idx[d, j] + 1, j] += frac[d, j]
    A = np.zeros((NDW, LL, NPRED), np.float64)
    w = np.asarray(conv_w, np.float64)
    if mode == "t":
        for p in range(NPRED):
            for k in range(3):
                j = p + k - 1
                if 0 <= j < NPRED:
                    A[:, :, p] += w[0, :, k][:, None] * M[:, :, j]
    else:
        for o in range(NPRED):
            for d in range(NDW):
                for c in range(NPRED):
                    A[d, :, o] += w[o, c, d] * M[d, :, c]
    return A.reshape(NDW * LL, NPRED)


def _plan(offset_t, offset_n, conv_t_w, conv_n_w, W):
    """Split used features into hi (fp16) / lo (fp8) index lists."""
    A_t = _build_A(offset_t, conv_t_w, "t")
    D = _build_A(offset_n, conv_n_w, "n") - A_t
    c2 = (A_t ** 2 + D ** 2).sum(axis=1)
    used = np.where(c2 > 0)[0]
    assert len(used) <= N_HI + N_LO, f"{len(used)} used features > {N_HI + N_LO}"
    n_hi = max(N_HI, len(used) - N_LO)
    by_weight = used[np.argsort(-c2[used])]
    hi = np.sort(by_weight[:n_hi])
    lo = np.sort(by_weight[n_hi:])
    lo_mask = np.ones(N_LO)
    lo_mask[len(lo):] = 0.0
    hi_mask = np.ones(N_HI)
    hi_mask[len(hi):] = 0.0
    hi = np.concatenate([hi, np.zeros(N_HI - len(hi), np.int64)])
    lo = np.concatenate([lo, np.zeros(N_LO - len(lo), np.int64)])
    Wf = np.asarray(W, np.float64)
    fold = np.linalg.cond(Wf) < MAX_W_COND
    return hi, hi_mask, lo, lo_mask, fold, A_t, D


def _build_weights(hi, hi_mask, lo, lo_mask, fold, A_t, D, W):
    """Stationary lhsT blocks packed as one [128, NW*112] tensor.

    block 0: gate blockdiag-W [96, 112]; block 1: hi lhsT [48, 112];
    block 2: lo lhsT [128, 112]; block 3: I48; block 4 (no-fold only):
    bias-permutation identity [96, 112]. x-path cols: Pt at 12g+j,
    Pd at 64+12g+j for group-slot g in 0..3.
    """
    NW = 5 if not fold else 4
    wts = np.zeros((128, NW * 112), np.float64)
    Wf = np.asarray(W, np.float64)
    for g in range(8):
        col = 12 * (g % 4) + 64 * (g // 4)
        wts[12 * g:12 * g + 12, col:col + 12] = Wf
        if not fold:
            for q in range(12):
                wts[12 * g + q, 4 * 112 + col + q] = 1.0
    A_hi = A_t[hi] * hi_mask[:, None]
    D_hi = D[hi] * hi_mask[:, None]
    A_lo = A_t[lo] * lo_mask[:, None]
    D_lo = D[lo] * lo_mask[:, None]
    for g in range(4):
        c = 112 + 12 * g
        wts[N_HI * g:N_HI * (g + 1), c:c + 12] = A_hi
        wts[N_HI * g:N_HI * (g + 1), c + 64:c + 76] = D_hi
        c = 224 + 12 * g
        wts[N_LO * g:N_LO * (g + 1), c:c + 12] = A_lo
        wts[N_LO * g:N_LO * (g + 1), c + 64:c + 76] = D_lo
    wts[0:48, 336:384] = np.eye(48)
    return wts.astype(np.float32)


def build_program(fold=True):
    import concourse.bass as bass
    import concourse.tile as tile
    from concourse import bacc, mybir
    from concourse.bass_interp import get_hw_module

    f32 = mybir.dt.float32
    f16 = mybir.dt.float16
    e3 = mybir.dt.float8e3
    NW = 4 if fold else 5

    nc = bacc.Bacc("TRN2", target_bir_lowering=False, debug=False,
                   num_devices=N_CORES)
    xhi = nc.dram_tensor("xhi", [BPC, 2, 4 * N_HI, 512], f16,
                         kind="ExternalInput").ap()
    xlo = nc.dram_tensor("xlo", [BPC, 2, 4 * N_LO, 512], e3,
                         kind="ExternalInput").ap()
    ct = nc.dram_tensor("ct", [BPC, 96, 512], f16, kind="ExternalInput").ap()
    wts = nc.dram_tensor("wts", [128, NW * 112], f16, kind="ExternalInput").ap()
    if not fold:
        bias = nc.dram_tensor("bias", [96, 512], f16, kind="ExternalInput").ap()
    yp = nc.dram_tensor("yp", [BPC, 96, 512], f16, kind="ExternalOutput").ap()

    with tile.TileContext(nc) as tc, ExitStack() as ctx:
        consts = ctx.enter_context(tc.tile_pool(name="consts", bufs=1))
        xpool = ctx.enter_context(tc.tile_pool(name="xp", bufs=1))
        cpool = ctx.enter_context(tc.tile_pool(name="ct", bufs=1))
        spool = ctx.enter_context(tc.tile_pool(name="sig", bufs=4))
        tpool = ctx.enter_context(tc.tile_pool(name="tmp", bufs=8))
        opool = ctx.enter_context(tc.tile_pool(name="ost", bufs=1))
        xps = ctx.enter_context(
            tc.tile_pool(name="xps", bufs=4, space=bass.MemorySpace.PSUM))
        gps = xps

        # ---- all input DMAs up-front, in pipeline order ----
        w_sb = consts.tile([128, NW * 112], f16)
        nc.sync.dma_start(w_sb[:], wts[:])
        if not fold:
            bias_sb = consts.tile([96, 512], f16)
            nc.sync.dma_start(bias_sb[:], bias[:])

        ct_tiles = [cpool.tile([96, 2048], f16, name=f"ct{i}") for i in range(4)]
        xhi_tiles = [None] * 8
        xlo_tiles = [None] * 8

        def load_ct(i4, half=None):
            # half granularity for the first tile (faster pipeline fill)
            b0 = i4 * 4 if half is None else i4 * 4 + half * 2
            nb = 4 if half is None else 2
            c0 = 0 if half in (None, 0) else 1024
            # ct goes through the Pool SWDGE path: no HWDGE hold, and the
            # GPSIMD engine is idle during the fill when these issue
            nc.gpsimd.dma_start(
                ct_tiles[i4][:, c0:c0 + nb * 512]
                .rearrange("p (b f) -> p b f", b=nb),
                ct[b0:b0 + nb].rearrange("b p f -> p b f"))

        def load_x(pair0, npair, split_batches=False):
            b0 = pair0 * 2
            thi = xpool.tile([4 * N_HI, npair * 2048], f16, name=f"xh{pair0}")
            tlo = xpool.tile([4 * N_LO, npair * 2048], e3, name=f"xl{pair0}")
            if split_batches:
                for b in range(2 * npair):
                    nc.sync.dma_start(
                        thi[:, b * 1024:(b + 1) * 1024]
                        .rearrange("p (h f) -> p h f", h=2),
                        xhi[b0 + b])
                    nc.sync.dma_start(
                        tlo[:, b * 1024:(b + 1) * 1024]
                        .rearrange("p (h f) -> p h f", h=2),
                        xlo[b0 + b])
            else:
                nc.sync.dma_start(
                    thi[:].rearrange("p (b h f) -> p b h f", b=2 * npair, h=2),
                    xhi[b0:b0 + 2 * npair].rearrange("b h p f -> p b h f"))
                nc.sync.dma_start(
                    tlo[:].rearrange("p (b h f) -> p b h f", b=2 * npair, h=2),
                    xlo[b0:b0 + 2 * npair].rearrange("b h p f -> p b h f"))
            for k in range(npair):
                xhi_tiles[pair0 + k] = (thi, k * 2048)
                xlo_tiles[pair0 + k] = (tlo, k * 2048)

        load_x(0, 1)
        load_ct(0, 0)
        load_x(1, 1)
        load_ct(0, 1)
        for i4 in range(1, 4):
            load_ct(i4)
            load_x(i4 * 2, 1)
            load_x(i4 * 2 + 1, 1)

        def w_blk(k, rows, ncols=112):
            return w_sb[rows, k * 112:k * 112 + ncols]

        # warm-up matmuls: keep PE busy through the DMA fill so the p-state
        # ramp completes before the first real matmul
        wp = xps.tile([112, 1024], f32, name="warm", tag="px", bufs=3)
        for i in range(N_WARM_MM):
            nc.tensor.matmul(wp[0:16, 0:WARM_FREE], w_sb[0:16, 0:16],
                             w_sb[0:16, 0:WARM_FREE], start=True, stop=True)


        # ---- per-pair stages; gates run one pair ahead ----
        def gates_stage(pair):
            i4, half = divmod(pair, 2)
            g_ps = gps.tile([112, 1024], f32, name=f"g{pair}", tag="px",
                            bufs=3)
            for b in range(2):
                cs = 1024 * half + 512 * b
                nc.tensor.matmul(g_ps[:, 512 * b:512 * b + 512],
                                 w_blk(0, slice(0, 96)),
                                 ct_tiles[i4][:, cs:cs + 512],
                                 start=True, stop=fold)
            if not fold:
                for b in range(2):
                    nc.tensor.matmul(g_ps[:, 512 * b:512 * b + 512],
                                     w_blk(4, slice(0, 96)), bias_sb[:],
                                     start=False, stop=True)
            return g_ps

        def sig_stage(g_ps, pair):
            s_sb = spool.tile([112, 1024], f16, name=f"s{pair}")
            nc.scalar.activation(s_sb[:], g_ps[:],
                                 mybir.ActivationFunctionType.Sigmoid)
            return s_sb

        def xpath_half(pair, px, h):
            thi, hoff = xhi_tiles[pair]
            tlo, loff = xlo_tiles[pair]
            for b in range(2):
                xc = (2 * b + h) * 512
                out = px[0][:, 512 * b:512 * b + 512] if h == 0 else px[1 + b][:]
                nc.tensor.matmul(out, w_blk(1, slice(0, 4 * N_HI)),
                                 thi[:, hoff + xc:hoff + xc + 512],
                                 start=True, stop=False)
                nc.tensor.matmul(out, w_blk(2, slice(0, 4 * N_LO)),
                                 tlo[:, loff + xc:loff + xc + 512],
                                 start=False, stop=True)

        def combine_d(pair, px, s_sb, o_sb, ocol):
            # quarter D (h1 cols cs:1024) first: its ACT copy only needs
            # the x-path, and the GPSIMD chain behind it is the longest
            d_sb = tpool.tile([112, 512], f16, name="d")
            nc.scalar.activation(d_sb[:], px[2][:],
                                 mybir.ActivationFunctionType.Copy)
            td = tpool.tile([48, 512], f16, name="td")
            nc.gpsimd.tensor_mul(td[:], d_sb[64:112, :],
                                 s_sb[64:112, 512:1024])
            nc.gpsimd.tensor_add(o_sb[64:112, ocol + 512:ocol + 1024],
                                 d_sb[0:48, :], td[:])

        def combine_rest(pair, px, s_sb, o_sb, ocol):
            # half h0: merged DVE mult, PE identity-adds, merged ACT copy
            t0 = tpool.tile([48, 1024], f16, name="t0")
            nc.vector.tensor_mul(t0[:], px[0][64:112, :], s_sb[0:48, :])
            for b in range(2):
                nc.tensor.matmul(px[0][0:48, 512 * b:512 * b + 512],
                                 w_blk(3, slice(0, 48), 48),
                                 t0[:, 512 * b:512 * b + 512],
                                 start=False, stop=True, skip_group_check=True)
            nc.scalar.activation(o_sb[0:48, ocol:ocol + 1024], px[0][0:48, :],
                                 mybir.ActivationFunctionType.Copy)
            # quarter C (h1 batch 0): fully on DVE, own single-bank tile
            t1 = tpool.tile([48, 512], f16, name="t1")
            nc.vector.tensor_mul(t1[:], px[1][64:112, :],
                                 s_sb[64:112, 0:512])
            nc.vector.tensor_add(o_sb[64:112, ocol:ocol + 512],
                                 px[1][0:48, :], t1[:])

        # per-pair output tiles: avoids cross-pair writer chaining on a
        # shared tile (the framework serializes same-tile accessors)
        o_tiles = [(opool.tile([112, 1024], f16, name=f"o{k}"), [k])
                   for k in range(8)]
        pair_otile = {}
        for o_sb, pairs in o_tiles:
            for i, p in enumerate(pairs):
                pair_otile[p] = (o_sb, i * 1024, p == pairs[-1])

        def store(o_sb, pairs):
            b0, nb = 2 * pairs[0], 2 * len(pairs)
            for r0, y0 in ((0, 0), (64, 48)):
                nc.sync.dma_start(
                    yp[b0:b0 + nb, y0:y0 + 48].rearrange("b p f -> p b f"),
                    o_sb[r0:r0 + 48, :].rearrange("p (b f) -> p b f", b=nb))

        s_store = {}
        g_store = {}
        for p in range(9):
            px = None
            if p >= 1:
                # allocation order [px0, gate, px1] makes the pool-slot reuse
                # edges land on early-freed tiles: px1(p) <- gate(p) (freed by
                # the sigmoid), gate(p+1) <- px0(p) (copy_h0, but gates have a
                # pair of lead slack), px0(p) <- px1(p-1) (freed mid-combine)
                px = [xps.tile([112, 1024], f32, name=f"px{p-1}_0",
                               tag="px", bufs=3),
                      None, None]
                xpath_half(p - 1, px, 0)
            if p < 8:
                g_store[p] = gates_stage(p)
            if p >= 1:
                px[1] = xps.tile([112, 512], f32, name=f"px{p-1}_C",
                                 tag="px2", bufs=2)
                px[2] = xps.tile([112, 512], f32, name=f"px{p-1}_D",
                                 tag="px2", bufs=2)
                xpath_half(p - 1, px, 1)
                o_sb, ocol, last = pair_otile[p - 1]
                if p == 1:
                    s_store[0] = sig_stage(g_store.pop(0), 0)
                s_prev = s_store.pop(p - 1)
                combine_d(p - 1, px, s_prev, o_sb, ocol)
                # sigmoid for pair p is emitted after the D-copy so the ACT
                # queue never head-blocks the GPSIMD feed
                if p < 8:
                    s_store[p] = sig_stage(g_store.pop(p), p)
                combine_rest(p - 1, px, s_prev, o_sb, ocol)
                if last:
                    store(*[ot for ot in o_tiles if p - 1 in ot[1]][0])

    nc.compile()
    nc.m = get_hw_module(nc.m)
    return nc


_PROGRAMS = {}


def _get_program(fold):
    if fold not in _PROGRAMS:
        _PROGRAMS[fold] = build_program(fold)
    return _PROGRAMS[fold]


def pack_inputs(inp, ctrl, bparam, W, hi, lo, fold):
    """Host-side shard + layout packing. Returns in_maps (list of 8 dicts)."""
    import ml_dtypes
    X = np.asarray(inp, np.float32).reshape(B, 2, 4, 512, NDW * LL)
    # [B, h, g', f, k] -> [B, h, g'*nk + k, f]
    Xhi = np.ascontiguousarray(
        X[..., hi].transpose(0, 1, 2, 4, 3)).reshape(B, 2, 4 * N_HI, 512)
    Xlo = np.ascontiguousarray(
        X[..., lo].transpose(0, 1, 2, 4, 3)).reshape(B, 2, 4 * N_LO, 512)
    ctf = np.asarray(ctrl, np.float64)
    if fold:
        binv = np.asarray(bparam, np.float64) @ np.linalg.inv(
            np.asarray(W, np.float64))
        ctf = ctf + binv[None, :, :]
    CT = np.ascontiguousarray(
        ctf.astype(np.float32).reshape(B, NG, 512, 12).transpose(0, 1, 3, 2)
    ).reshape(B, 96, 512)
    Xhi = Xhi.astype(np.float16)
    Xlo = Xlo.astype(ml_dtypes.float8_e3m4)
    CT = CT.astype(np.float16)
    in_maps = []
    for c in range(N_CORES):
        sl = slice(c * BPC, (c + 1) * BPC)
        in_maps.append({"xhi": Xhi[sl], "xlo": Xlo[sl], "ct": CT[sl]})
    return in_maps


def unpack_output(results):
    yp = np.concatenate([r["yp"].astype(np.float32) for r in results], axis=0)
    return np.ascontiguousarray(
        yp.reshape(B, NG, 12, 512).transpose(0, 1, 3, 2)
    ).reshape(B, NNODES, NPRED)


def kernel(inp, ctrl, offset_t, offset_n, conv_t_w, conv_t_b, conv_n_w,
           conv_n_b, W, bparam):
    from concourse.bass_utils import run_bass_kernel_spmd

    hi, hi_mask, lo, lo_mask, fold, A_t, D = _plan(
        offset_t, offset_n, conv_t_w, conv_n_w, W)
    nc = _get_program(fold)
    wts_np = _build_weights(hi, hi_mask, lo, lo_mask, fold, A_t, D, W)
    in_maps = pack_inputs(inp, ctrl, bparam, W, hi, lo, fold)
    for m in in_maps:
        m["wts"] = wts_np.astype(np.float16)
        if not fold:
            bias_t = np.ascontiguousarray(
                np.asarray(bparam, np.float32).reshape(NG, 512, 12)
                .transpose(0, 2, 1)).reshape(96, 512)
            m["bias"] = bias_t.astype(np.float16)
    res = run_bass_kernel_spmd(nc, in_maps, core_ids=list(range(N_CORES)))
    out = unpack_output(res.results)
    # Conv biases are zeros in this module's init, so the device kernel omits
    # them. If ever nonzero, apply the exact correction on the host.
    ctb = float(np.asarray(conv_t_b).reshape(-1)[0])
    cnb = np.asarray(conv_n_b, np.float32)
    if ctb != 0.0 or np.any(cnb != 0.0):
        G = np.asarray(ctrl, np.float32).reshape(B * NNODES, NPRED) @ np.asarray(
            W, np.float32)
        G += np.tile(np.asarray(bparam, np.float32), (B, 1))
        S = 1.0 / (1.0 + np.exp(-G))
        out = out + (ctb + (cnb[None, :] - ctb) * S).reshape(B, NNODES, NPRED)
    return out.astype(np.float32)
